# revision 1
# baseline (speedup 1.0000x reference)
"""Distributed FNO block on 8 TRN2 NeuronCores.

Strategy: batch-parallel (B=8 -> one batch element per core) for the channel
mixer and both spatial DFTs; mode-parallel (ky sharded, 4 modes per core) for
the spectral channel mixing, with an AllToAll in each direction.

All DFTs are truncated-mode DFT matmuls (only 64 kx x 32 ky modes survive),
computed in bf16 on the TensorEngine with fp32 PSUM accumulation.

Self-contained: shapes/sharding hardcoded, no sibling imports.
"""
import numpy as np
import ml_dtypes
from contextlib import ExitStack

import concourse.bass as bass
import concourse.bacc as bacc
import concourse.tile as tile
from concourse import mybir
from concourse.bass_utils import run_bass_kernel_spmd

B, C, H, W = 8, 128, 256, 256
M0, M1 = 32, 32
NCORES = 8
KX = np.concatenate([np.arange(32), np.arange(H - 32, H)])  # 64 kept kx modes
BF = mybir.dt.bfloat16
F32 = mybir.dt.float32
BF_NP = ml_dtypes.bfloat16


# ----------------------------------------------------------------- host consts
def _consts():
    h = np.arange(H)[:, None]
    w = np.arange(W)[:, None]
    ky = np.arange(M1)
    th = 2 * np.pi * h * KX[None, :] / H
    FH = np.concatenate([np.cos(th), -np.sin(th)], axis=1)  # [256, 128]
    tw = 2 * np.pi * w * ky[None, :] / W
    FW = np.concatenate([np.cos(tw), -np.sin(tw)], axis=1)  # [256, 64]
    thi = 2 * np.pi * np.arange(H)[None, :] * KX[:, None] / H  # [64, 256]
    GH1 = np.concatenate([np.cos(thi) / H, -np.sin(thi) / H], axis=0)  # [128,256]
    GH2 = np.concatenate([np.sin(thi) / H, np.cos(thi) / H], axis=0)   # [128,256]
    twi = 2 * np.pi * ky[:, None] * np.arange(W)[None, :] / W  # [32, 256]
    wt = np.where(ky == 0, 1.0, 2.0)[:, None]
    CW = np.concatenate([wt * np.cos(twi) / W, -wt * np.sin(twi) / W], axis=0)
    CW[32, :] = 0.0  # irfft drops Im(Y[ky=0])
    return (FH.astype(BF_NP), FW.astype(BF_NP), GH1.astype(BF_NP),
            GH2.astype(BF_NP), CW.astype(BF_NP))


def _wmix_slices(w1r, w1i, w2r, w2i):
    """Per-core spectral weight slice [4ky_in, 64kx, 2ri, 128ci, 128co] bf16."""
    # full [32ky, 64kx, 2, ci, co]
    wr = np.concatenate([w1r, w2r], axis=2)  # [ci, co, 64kx, 32ky]
    wi = np.concatenate([w1i, w2i], axis=2)
    wall = np.stack([wr, wi], axis=0)  # [2, ci, co, kx, ky]
    wall = wall.transpose(4, 3, 0, 1, 2)  # [ky, kx, 2, ci, co]
    wall = np.ascontiguousarray(wall).astype(BF_NP)
    return [np.ascontiguousarray(wall[4 * k:4 * k + 4]) for k in range(NCORES)]


# ----------------------------------------------------------------- bass kernel
def _build_nc():
    nc = bacc.Bacc(num_devices=NCORES)

    x_d = nc.declare_dram_parameter("x", [C, H, W], BF, isOutput=False)
    wlt_d = nc.declare_dram_parameter("wlt", [C, C], BF, isOutput=False)
    fh_d = nc.declare_dram_parameter("fh", [H, 128], BF, isOutput=False)
    fw_d = nc.declare_dram_parameter("fw", [W, 64], BF, isOutput=False)
    gh1_d = nc.declare_dram_parameter("gh1", [128, H], BF, isOutput=False)
    gh2_d = nc.declare_dram_parameter("gh2", [128, H], BF, isOutput=False)
    cw_d = nc.declare_dram_parameter("cw", [64, W], BF, isOutput=False)
    wmix_d = nc.declare_dram_parameter("wmix", [4, 64, 2, C, C], BF, isOutput=False)
    out_d = nc.declare_dram_parameter("out", [C, H, W], F32, isOutput=True)

    # internal DRAM
    y0s = nc.dram_tensor("y0s", [C, H, W], BF)
    send1 = nc.dram_tensor("send1", [8, 4, 2, C, 64], BF)
    recv1 = nc.dram_tensor("recv1", [8, 4, 2, C, 64], BF)
    send2 = nc.dram_tensor("send2", [8, 4, C, 2, 64], BF)
    recv2 = nc.dram_tensor("recv2", [8, 4, C, 2, 64], BF)

    rg = [list(range(NCORES))]

    with tile.TileContext(nc) as tc, ExitStack() as ctx:
        cpool = ctx.enter_context(tc.tile_pool(name="consts", bufs=1))
        spool = ctx.enter_context(tc.tile_pool(name="stages", bufs=1))
        xpool = ctx.enter_context(tc.tile_pool(name="x", bufs=8))
        xhpool = ctx.enter_context(tc.tile_pool(name="xh", bufs=8))
        ypool = ctx.enter_context(tc.tile_pool(name="y", bufs=6))
        wpool = ctx.enter_context(tc.tile_pool(name="wmix", bufs=16))
        zpool = ctx.enter_context(tc.tile_pool(name="z", bufs=4))
        opool = ctx.enter_context(tc.tile_pool(name="o", bufs=6))
        psy_ctx = ExitStack()
        psy_p = psy_ctx.enter_context(
            tc.tile_pool(name="psy", bufs=2, space="PSUM"))

        # constants into SBUF
        fh_sb = [cpool.tile([128, 128], BF, tag=f"fh{t}", name=f"fh{t}")
                 for t in range(2)]
        fw_sb = [cpool.tile([128, 64], BF, tag=f"fw{t}", name=f"fw{t}")
                 for t in range(2)]
        for t in range(2):
            nc.sync.dma_start(fh_sb[t][:], fh_d[128 * t:128 * (t + 1), :])
            nc.sync.dma_start(fw_sb[t][:], fw_d[128 * t:128 * (t + 1), :])
        wlt_sb = cpool.tile([C, C], BF, tag="wlt")
        nc.sync.dma_start(wlt_sb[:], wlt_d[:])
        gh1_sb = cpool.tile([128, H], BF, tag="gh1")
        gh2_sb = cpool.tile([128, H], BF, tag="gh2")
        cw_sb = cpool.tile([64, W], BF, tag="cw")
        nc.sync.dma_start(gh1_sb[:], gh1_d[:])
        nc.sync.dma_start(gh2_sb[:], gh2_d[:])
        nc.sync.dma_start(cw_sb[:], cw_d[:])

        # big staging tiles
        mm_in = spool.tile([C, 8, 4, 2, 64], BF, tag="mm_in")  # A2A#1 recv
        rhs1 = spool.tile([C, 4, 64, 2, 8], BF, tag="rhs1")    # modemix rhs
        rhs2 = spool.tile([C, 4, 64, 2, 8], BF, tag="rhs2")
        stage2 = spool.tile([C, 8, 4, 2, 64], BF, tag="stage2")  # modemix out
        inv2 = spool.tile([128, 8, 4, C], BF, tag="inv2")      # A2A#2 recv

        # ---------------- forward truncated DFT (per channel) ----------------
        with tc.tile_pool(name="psA", bufs=3, space="PSUM") as psA_p, \
             tc.tile_pool(name="ps2", bufs=3, space="PSUM") as ps2_p:
            stage1R = spool.tile([32, C, 64], BF, tag="stage1R")
            stage1I = spool.tile([32, C, 64], BF, tag="stage1I")
            for c in range(C):
                # single DMA per channel: [h128, (ht2, w256)]
                xrow = xpool.tile([128, 2, 256], BF, tag="xt", name="xt")
                nc.sync.dma_start(
                    xrow[:], x_d[c].rearrange("(t h) w -> h t w", t=2))
                xh = [None, None]
                for wt_i in range(2):
                    ps = psA_p.tile([128, 128], F32, tag="psA")
                    for ht in range(2):
                        nc.tensor.matmul(
                            ps[:], xrow[:, ht, 128 * wt_i:128 * (wt_i + 1)],
                            fh_sb[ht][:], start=(ht == 0), stop=(ht == 1))
                    xh[wt_i] = xhpool.tile([128, 128], BF, tag="xh", name="xh")
                    nc.vector.tensor_copy(xh[wt_i][:], ps[:])
                ps2 = ps2_p.tile([64, 128], F32, tag="ps2")
                for wt_i in range(2):
                    nc.tensor.matmul(ps2[:], fw_sb[wt_i][:], xh[wt_i][:],
                                     start=(wt_i == 0), stop=(wt_i == 1))
                # complex combine; separate base-0 tiles (TensorTensor needs
                # equal base partitions for both SBUF inputs)
                xsA = xhpool.tile([32, 128], F32, tag="xsA")
                xsB = xhpool.tile([32, 128], F32, tag="xsB")
                nc.vector.tensor_copy(xsA[:], ps2[0:32, :])
                nc.vector.tensor_copy(xsB[:], ps2[32:64, :])
                nc.vector.tensor_sub(stage1R[:, c, :], xsA[:, 0:64],
                                     xsB[:, 64:128])
                nc.vector.tensor_add(stage1I[:, c, :], xsB[:, 0:64],
                                     xsA[:, 64:128])

        # A2A #1: ky-shard the spectrum
        for g in range(8):
            nc.sync.dma_start(send1[g, :, 0, :, :], stage1R[4 * g:4 * g + 4, :, :])
            nc.sync.dma_start(send1[g, :, 1, :, :], stage1I[4 * g:4 * g + 4, :, :])
        nc.gpsimd.collective_compute(
            "AllToAll", mybir.AluOpType.bypass, replica_groups=rg,
            ins=[send1[:].opt()], outs=[recv1[:].opt()])

        # ---------------- y0 = W_lin @ x (channel mixer), independent --------
        for t in range(64):
            xt = ypool.tile([C, 4, W], BF, tag="yx")
            nc.gpsimd.dma_start(xt[:], x_d[:, 4 * t:4 * t + 4, :])
            y0t = ypool.tile([C, 4, W], BF, tag="y0t")
            for j in range(2):
                psy = psy_p.tile([C, 2, W], F32, tag="psy")
                nc.tensor.matmul(psy[:], wlt_sb[:], xt[:, 2 * j:2 * j + 2, :])
                nc.scalar.copy(y0t[:, 2 * j:2 * j + 2, :], psy[:])
            nc.gpsimd.dma_start(y0s[:, 4 * t:4 * t + 4, :], y0t[:])

        # ---------------- modemix (ky-sharded, all batches) ------------------
        nc.sync.dma_start(mm_in[:], recv1[:].rearrange("b k r c x -> c b k r x"))
        # rhs1 = [XsR | XsI], rhs2 = [-XsI | XsR] per mode, cols (ri_half, b)
        nc.vector.tensor_copy(rhs1[:, :, :, 0, :],
                              mm_in[:].rearrange("c b k r x -> c k x r b")[:, :, :, 0, :])
        nc.vector.tensor_copy(rhs1[:, :, :, 1, :],
                              mm_in[:].rearrange("c b k r x -> c k x r b")[:, :, :, 1, :])
        nc.vector.tensor_scalar_mul(
            rhs2[:, :, :, 0, :],
            mm_in[:].rearrange("c b k r x -> c k x r b")[:, :, :, 1, :], -1.0)
        nc.vector.tensor_copy(rhs2[:, :, :, 1, :],
                              mm_in[:].rearrange("c b k r x -> c k x r b")[:, :, :, 0, :])

        with tc.tile_pool(name="psm", bufs=6, space="PSUM") as psm_p:
            for kyi in range(4):
                for kxb in range(16):  # blocks of 4 kx modes
                    # one big prefetchable weight block [ci, 4kx, 2ri, co]
                    wblk = wpool.tile([C, 4, 2, C], BF, tag="wblk")
                    nc.gpsimd.dma_start(
                        wblk[:],
                        wmix_d[kyi, 4 * kxb:4 * kxb + 4].rearrange(
                            "k r c o -> c k r o"))
                    for kxi in range(4):
                        kx = 4 * kxb + kxi
                        psm = psm_p.tile([C, 2, 8], F32, tag="psm")
                        nc.tensor.matmul(psm[:], wblk[:, kxi, 0, :],
                                         rhs1[:, kyi, kx, :, :],
                                         start=True, stop=False)
                        nc.tensor.matmul(psm[:], wblk[:, kxi, 1, :],
                                         rhs2[:, kyi, kx, :, :],
                                         start=False, stop=True)
                        nc.vector.tensor_copy(stage2[:, :, kyi, :, kx],
                                              psm[:].rearrange("c r b -> c b r"))

        # A2A #2: back to batch-sharded full spectrum
        for b in range(8):
            nc.sync.dma_start(
                send2[b].rearrange("k c r x -> c k r x"), stage2[:, b])
        nc.gpsimd.collective_compute(
            "AllToAll", mybir.AluOpType.bypass, replica_groups=rg,
            ins=[send2[:].opt()], outs=[recv2[:].opt()])

        # ---------------- inverse transforms + y0 add ------------------------
        # xbar transpose: [(g kyin co), (ri kx)] -> [(ri kx), (g kyin co)]
        nc.sync.dma_start_transpose(
            inv2[:], recv2[:].rearrange("g k c r x -> (g k c) (r x)"))

        psy_ctx.close()  # free y0 psum banks for the inverse pools
        with tc.tile_pool(name="psZ", bufs=2, space="PSUM") as psZ_p, \
             tc.tile_pool(name="psO", bufs=4, space="PSUM") as psO_p:
            for co in range(C):
                # both Z halves in one psum bank via col-group tiling
                psZ = psZ_p.tile([64, H], F32, tag="psZ")
                nc.tensor.matmul(psZ[0:32, :], inv2[:, :, :, co], gh1_sb[:],
                                 tile_position=(0, 0))
                nc.tensor.matmul(psZ[32:64, :], inv2[:, :, :, co], gh2_sb[:],
                                 tile_position=(0, 32))
                z_sb = zpool.tile([64, H], BF, tag="z")
                nc.vector.tensor_copy(z_sb[:], psZ[:])
                # single-DMA y0 load and out store per channel
                y0t = opool.tile([128, 2, W], BF, tag="oy0")
                nc.scalar.dma_start(
                    y0t[:], y0s[co].rearrange("(t h) w -> h t w", t=2))
                outt = opool.tile([128, 2, W], F32, tag="outt")
                for ht in range(2):
                    psO = psO_p.tile([128, W], F32, tag="psO")
                    nc.tensor.matmul(psO[:], z_sb[:, 128 * ht:128 * (ht + 1)],
                                     cw_sb[:])
                    nc.vector.tensor_add(outt[:, ht, :], psO[:],
                                         y0t[:, ht, :])
                nc.sync.dma_start(
                    out_d[co].rearrange("(t h) w -> h t w", t=2), outt[:])

    nc.compile()
    return nc


_NC_CACHE = {}


def kernel(x, W_lin, w1r, w1i, w2r, w2i):
    x = np.asarray(x)
    FH, FW, GH1, GH2, CW = _consts()
    wlt = np.ascontiguousarray(np.asarray(W_lin).T).astype(BF_NP)
    wmix = _wmix_slices(np.asarray(w1r), np.asarray(w1i),
                        np.asarray(w2r), np.asarray(w2i))

    if "nc" not in _NC_CACHE:
        _NC_CACHE["nc"] = _build_nc()
    nc = _NC_CACHE["nc"]

    in_maps = []
    for k in range(NCORES):
        in_maps.append({
            "x": np.ascontiguousarray(x[k]).astype(BF_NP),
            "wlt": wlt, "fh": FH, "fw": FW,
            "gh1": GH1, "gh2": GH2, "cw": CW,
            "wmix": wmix[k],
        })
    res = run_bass_kernel_spmd(nc, in_maps, list(range(NCORES)))
    out = np.stack([res.results[k]["out"] for k in range(NCORES)], axis=0)
    return out.astype(np.float32)



# revision 6
# speedup vs baseline: 1.1523x; 1.1523x over previous
"""Distributed FNO block on 8 TRN2 NeuronCores — v2.

Strategy: batch-parallel (B=8 -> one batch element per core) for the channel
mixer and both spatial DFTs; mode-parallel (ky sharded, 4 modes per core) for
the spectral channel mixing, with an AllToAll in each direction.

v2 changes vs baseline:
  - y0 (channel mixer) writes straight to out_d; the inverse stage
    accumulates into it with SWDGE dma accum (kills the y0s roundtrip).
  - out_d is bf16 (host casts back to f32).
  - y0 compute is issued AFTER the first AllToAll so it fills the
    collective's dead window on the TensorE queue.
  - wmix is stored host-side in a DMA-contiguous layout and streamed on the
    scalar HWDGE queue, prefetched from early in the kernel.
  - forward W-DFT splits into two PSUM banks so the complex combine reads
    PSUM directly (2 DVE ops/channel instead of 4).
  - inverse processes channel pairs with tile_position packing (full
    128-partition PSUM tiles, concurrent sub-array matmuls).
  - engine rebalance: ACT evacuates PSUM, GpSimd does the mode-mix
    rearranges and collectives, sync/scalar issue all bulk DMA (HWDGE).

Self-contained: shapes/sharding hardcoded, no sibling imports.
"""
import numpy as np
import ml_dtypes
from contextlib import ExitStack

import concourse.bass as bass
import concourse.bacc as bacc
import concourse.tile as tile
from concourse import mybir
from concourse.bass_utils import run_bass_kernel_spmd

B, C, H, W = 8, 128, 256, 256
M0, M1 = 32, 32
NCORES = 8
KX = np.concatenate([np.arange(32), np.arange(H - 32, H)])  # 64 kept kx modes
BF = mybir.dt.bfloat16
F32 = mybir.dt.float32
BF_NP = ml_dtypes.bfloat16


# ----------------------------------------------------------------- host consts
def _consts():
    h = np.arange(H)[:, None]
    w = np.arange(W)[:, None]
    ky = np.arange(M1)
    th = 2 * np.pi * h * KX[None, :] / H
    FH = np.concatenate([np.cos(th), -np.sin(th)], axis=1)  # [256, 128]
    tw = 2 * np.pi * w * ky[None, :] / W
    # [cos | -sin | +sin] so Sr/Si accumulate directly in PSUM
    FW = np.concatenate([np.cos(tw), -np.sin(tw), np.sin(tw)], axis=1)  # [256,96]
    thi = 2 * np.pi * np.arange(H)[None, :] * KX[:, None] / H  # [64, 256]
    GH1 = np.concatenate([np.cos(thi) / H, -np.sin(thi) / H], axis=0)  # [128,256]
    GH2 = np.concatenate([np.sin(thi) / H, np.cos(thi) / H], axis=0)   # [128,256]
    twi = 2 * np.pi * ky[:, None] * np.arange(W)[None, :] / W  # [32, 256]
    wt = np.where(ky == 0, 1.0, 2.0)[:, None]
    CW = np.concatenate([wt * np.cos(twi) / W, -wt * np.sin(twi) / W], axis=0)
    CW[32, :] = 0.0  # irfft drops Im(Y[ky=0])
    # duplicated into both partition halves for the row-tiled inverse-W matmul
    CW2 = np.concatenate([CW, CW], axis=0)  # [128, 256]
    return (FH.astype(BF_NP), FW.astype(BF_NP), GH1.astype(BF_NP),
            GH2.astype(BF_NP), CW2.astype(BF_NP))


def _wmix_slices(w1r, w1i, w2r, w2i):
    """Per-core spectral weight slice, DMA-contiguous for the wblk tiles.

    Layout [4ky, 8kxb, ci, 8kx, 2ri, co] so one 512KB DMA fills a
    [C, 8, 2, C] SBUF tile contiguously per partition."""
    wr = np.concatenate([w1r, w2r], axis=2)  # [ci, co, 64kx, 32ky]
    wi = np.concatenate([w1i, w2i], axis=2)
    wall = np.stack([wr, wi], axis=0)  # [ri, ci, co, kx, ky]
    # -> [ky, kxb, ci, kxi, ri, co]
    wall = wall.reshape(2, C, C, 8, 8, 32)
    wall = wall.transpose(5, 3, 1, 4, 0, 2)  # ky, kxb, ci, kxi, ri, co
    wall = np.ascontiguousarray(wall).astype(BF_NP)
    return [np.ascontiguousarray(wall[4 * k:4 * k + 4]) for k in range(NCORES)]


# ----------------------------------------------------------------- bass kernel
def _build_nc():
    nc = bacc.Bacc(num_devices=NCORES)

    x_d = nc.declare_dram_parameter("x", [C, H, W], BF, isOutput=False)
    wlt_d = nc.declare_dram_parameter("wlt", [C, C], BF, isOutput=False)
    fh_d = nc.declare_dram_parameter("fh", [H, 128], BF, isOutput=False)
    fw_d = nc.declare_dram_parameter("fw", [W, 96], BF, isOutput=False)
    gh1_d = nc.declare_dram_parameter("gh1", [128, H], BF, isOutput=False)
    gh2_d = nc.declare_dram_parameter("gh2", [128, H], BF, isOutput=False)
    cw_d = nc.declare_dram_parameter("cw", [128, W], BF, isOutput=False)
    wmix_d = nc.declare_dram_parameter("wmix", [4, 8, C, 8, 2, C], BF,
                                       isOutput=False)
    out_d = nc.declare_dram_parameter("out", [C, H, W], BF, isOutput=True)

    # internal DRAM
    send1 = nc.dram_tensor("send1", [8, C, 4, 2, 64], BF)
    recv1 = nc.dram_tensor("recv1", [8, C, 4, 2, 64], BF)
    send2 = nc.dram_tensor("send2", [8, 4, C, 2, 64], BF)
    recv2 = nc.dram_tensor("recv2", [8, 4, C, 2, 64], BF)

    rg = [list(range(NCORES))]
    A = mybir.AluOpType

    with tile.TileContext(nc) as tc, ExitStack() as ctx:
        cpool = ctx.enter_context(tc.tile_pool(name="consts", bufs=1))
        spool = ctx.enter_context(tc.tile_pool(name="stages", bufs=1))
        xqpool = ctx.enter_context(tc.tile_pool(name="xq", bufs=6))
        xhpool = ctx.enter_context(tc.tile_pool(name="xh", bufs=6))
        ypool = ctx.enter_context(tc.tile_pool(name="y", bufs=4))
        wpool = ctx.enter_context(tc.tile_pool(name="wmix", bufs=6))
        zpool = ctx.enter_context(tc.tile_pool(name="z", bufs=4))
        opool = ctx.enter_context(tc.tile_pool(name="o", bufs=4))

        # constants into SBUF
        fh_sb = [cpool.tile([128, 128], BF, tag=f"fh{t}", name=f"fh{t}")
                 for t in range(2)]
        fw_sb = [cpool.tile([128, 96], BF, tag=f"fw{t}", name=f"fw{t}")
                 for t in range(2)]
        for t in range(2):
            nc.sync.dma_start(fh_sb[t][:], fh_d[128 * t:128 * (t + 1), :])
            nc.sync.dma_start(fw_sb[t][:], fw_d[128 * t:128 * (t + 1), :])
        wlt_sb = cpool.tile([C, C], BF, tag="wlt")
        nc.sync.dma_start(wlt_sb[:], wlt_d[:])
        gh1_sb = cpool.tile([128, H], BF, tag="gh1")
        gh2_sb = cpool.tile([128, H], BF, tag="gh2")
        cw_sb = cpool.tile([128, W], BF, tag="cw")
        nc.sync.dma_start(gh1_sb[:], gh1_d[:])
        nc.sync.dma_start(gh2_sb[:], gh2_d[:])
        nc.sync.dma_start(cw_sb[:], cw_d[:])

        # big staging tiles
        stage1 = spool.tile([64, C, 64], BF, tag="stage1")  # [(ri ky), c, kx]
        mm_in = spool.tile([C, 8, 4, 2, 64], BF, tag="mm_in")  # A2A#1 recv
        rhs1 = spool.tile([C, 4, 64, 2, 8], BF, tag="rhs1")    # modemix rhs
        rhs2 = spool.tile([C, 4, 64, 2, 8], BF, tag="rhs2")
        stage2 = spool.tile([C, 8, 4, 2, 64], BF, tag="stage2")  # modemix out
        inv2 = spool.tile([128, 8, 4, C], BF, tag="inv2")      # A2A#2 recv

        # wmix streaming: 6 blocks prefetch upfront (SWDGE, gpsimd queue),
        # the rest are issued just-in-time inside the mix loop so pool
        # recycling never stalls a queue that later phases depend on.
        wblk_tiles = [None] * 32

        def issue_wblk(i):
            kyi, kxb = divmod(i, 8)
            t = wpool.tile([C, 8, 2, C], BF, tag="wblk", name="wblk")
            nc.gpsimd.dma_start(t[:], wmix_d[kyi, kxb])
            wblk_tiles[i] = t

        for i in range(6):
            issue_wblk(i)

        # ---------------- forward truncated DFT (per channel) ----------------
        with tc.tile_pool(name="psA", bufs=4, space="PSUM") as psA_p, \
             tc.tile_pool(name="psW", bufs=4, space="PSUM") as psW_p:
            for blk in range(32):
                xq = xqpool.tile([128, 4, 2, 256], BF, tag="xq", name="xq")
                nc.sync.dma_start(
                    xq[:], x_d[4 * blk:4 * blk + 4].rearrange(
                        "c (t h) w -> h c t w", t=2))
                for ci in range(4):
                    c = 4 * blk + ci
                    xh = [None, None]
                    for wt_i in range(2):
                        ps = psA_p.tile([128, 128], F32, tag="psA",
                                        padded_shape=(None, 512))
                        for ht in range(2):
                            nc.tensor.matmul(
                                ps[:],
                                xq[:, ci, ht, 128 * wt_i:128 * (wt_i + 1)],
                                fh_sb[ht][:], start=(ht == 0), stop=(ht == 1))
                        xh[wt_i] = xhpool.tile([128, 128], BF, tag="xh",
                                               name="xh")
                        if wt_i == 0:
                            nc.scalar.copy(xh[wt_i][:], ps[:])
                        else:
                            nc.vector.tensor_copy(xh[wt_i][:], ps[:])
                    # Sr = cos@Xr + sin@Xi (rows 0:32), Si = cos@Xi - sin@Xr
                    # (rows 32:64), accumulated in PSUM over both w-halves.
                    psRI = psW_p.tile([64, 64], F32, tag="psRI", name="psRI",
                                      padded_shape=(None, 512))
                    for wt_i in range(2):
                        st = wt_i == 0
                        sp = wt_i == 1
                        xr = xh[wt_i][:, 0:64]
                        xi = xh[wt_i][:, 64:128]
                        fw = fw_sb[wt_i]
                        nc.tensor.matmul(psRI[0:32, :], fw[:, 0:32], xr,
                                         start=st, stop=False,
                                         tile_position=(0, 0))
                        nc.tensor.matmul(psRI[32:64, :], fw[:, 0:32], xi,
                                         start=st, stop=False,
                                         tile_position=(0, 32))
                        nc.tensor.matmul(psRI[0:32, :], fw[:, 64:96], xi,
                                         start=False, stop=sp,
                                         tile_position=(0, 0))
                        nc.tensor.matmul(psRI[32:64, :], fw[:, 32:64], xr,
                                         start=False, stop=sp,
                                         tile_position=(0, 32))
                    nc.vector.tensor_copy(stage1[:, c, :], psRI[:])

        # A2A #1: ky-shard the spectrum (c-major payload)
        for g in range(8):
            for r in range(2):
                nc.sync.dma_start(
                    send1[g][:, :, r, :].rearrange("c k x -> k c x"),
                    stage1[32 * r + 4 * g:32 * r + 4 * g + 4])
        nc.gpsimd.collective_compute(
            "AllToAll", A.bypass, replica_groups=rg,
            ins=[send1[:].opt()], outs=[recv1[:].opt()])

        # mm_in + rhs prep (gpsimd tensor ops; queue order after A2A#1)
        nc.sync.dma_start(mm_in[:], recv1[:].rearrange("b c k r x -> c b k r x"))
        mm_v = mm_in[:].rearrange("c b k r x -> c k x r b")
        nc.gpsimd.tensor_copy(rhs1[:, :, :, 0, :], mm_v[:, :, :, 0, :])
        nc.gpsimd.tensor_copy(rhs1[:, :, :, 1, :], mm_v[:, :, :, 1, :])
        nc.gpsimd.tensor_scalar_mul(rhs2[:, :, :, 0, :], mm_v[:, :, :, 1, :],
                                    -1.0)
        nc.gpsimd.tensor_copy(rhs2[:, :, :, 1, :], mm_v[:, :, :, 0, :])

        # ---------------- y0 = W_lin @ x -> out_d (fills A2A#1 window) -------
        with tc.tile_pool(name="psy", bufs=2, space="PSUM") as psy_p, \
             tc.tile_pool(name="psm", bufs=4, space="PSUM") as psm_p:
            def y0_iter(t):
                xt = ypool.tile([C, 8, W], BF, tag="yx", name="yx")
                nc.sync.dma_start(xt[:], x_d[:, 8 * t:8 * t + 8, :])
                y0t = ypool.tile([C, 8, W], BF, tag="y0t", name="y0t")
                for j in range(4):
                    psy = psy_p.tile([C, 2, W], F32, tag="psy")
                    nc.tensor.matmul(psy[:], wlt_sb[:],
                                     xt[:, 2 * j:2 * j + 2, :])
                    if j % 2 == 0:
                        nc.vector.tensor_copy(y0t[:, 2 * j:2 * j + 2, :],
                                              psy[:])
                    else:
                        nc.scalar.copy(y0t[:, 2 * j:2 * j + 2, :], psy[:])
                nc.sync.dma_start(out_d[:, 8 * t:8 * t + 8, :], y0t[:])

            for t in range(24):
                y0_iter(t)

            # ---------------- modemix (ky-sharded, all batches) --------------
            for kyi in range(4):
                for kxb in range(8):
                    i = 8 * kyi + kxb
                    if i + 6 < 32:
                        issue_wblk(i + 6)
                    wblk = wblk_tiles[i]
                    wblk_tiles[i] = None
                    for half in range(2):
                        psm = psm_p.tile([C, 4, 2, 8], F32, tag="psm",
                                         padded_shape=(None, None, None, 64))
                        for kxi in range(4):
                            slot = 4 * half + kxi
                            kx = 8 * kxb + slot
                            nc.tensor.matmul(psm[:, kxi], wblk[:, slot, 0, :],
                                             rhs1[:, kyi, kx, :, :],
                                             start=True, stop=False)
                            nc.tensor.matmul(psm[:, kxi], wblk[:, slot, 1, :],
                                             rhs2[:, kyi, kx, :, :],
                                             start=False, stop=True)
                        kx0 = 8 * kxb + 4 * half
                        nc.vector.tensor_copy(
                            stage2[:, :, kyi, :, kx0:kx0 + 4],
                            psm[:].rearrange("c x r b -> c b r x"))

            # A2A #2: back to batch-sharded full spectrum
            for b in range(8):
                nc.sync.dma_start(
                    send2[b].rearrange("k c r x -> c k r x"), stage2[:, b])
            nc.gpsimd.collective_compute(
                "AllToAll", A.bypass, replica_groups=rg,
                ins=[send2[:].opt()], outs=[recv2[:].opt()])

            # y0 tail fills the A2A#2 window (issued before the transpose so
            # the sync queue doesn't stall behind the recv2 wait)
            for t in range(24, 32):
                y0_iter(t)
            nc.sync.dma_start_transpose(
                inv2[:], recv2[:].rearrange("g k c r x -> (g k c) (r x)"))

        # ---------------- inverse transforms, accumulate into out_d ----------
        with tc.tile_pool(name="psZ", bufs=2, space="PSUM") as psZ_p, \
             tc.tile_pool(name="psO", bufs=4, space="PSUM") as psO_p:
            for cp in range(64):
                co = 2 * cp
                # 4 concurrent col-tiled matmuls: [Yr(co);Yi(co);Yr(co');Yi(co')]
                psZ = psZ_p.tile([128, H], F32, tag="psZ",
                                 padded_shape=(None, 512))
                nc.tensor.matmul(psZ[0:32, :], inv2[:, :, :, co], gh1_sb[:],
                                 tile_position=(0, 0))
                nc.tensor.matmul(psZ[32:64, :], inv2[:, :, :, co], gh2_sb[:],
                                 tile_position=(0, 32))
                nc.tensor.matmul(psZ[64:96, :], inv2[:, :, :, co + 1],
                                 gh1_sb[:], tile_position=(0, 64))
                nc.tensor.matmul(psZ[96:128, :], inv2[:, :, :, co + 1],
                                 gh2_sb[:], tile_position=(0, 96))
                z_sb = zpool.tile([128, H], BF, tag="z")
                nc.vector.tensor_copy(z_sb[:], psZ[:])
                outt = opool.tile([128, 2, 2, W], BF, tag="outt")
                for p in range(2):
                    psO = psO_p.tile([128, 2, W], F32, tag="psO")
                    for ht in range(2):
                        nc.tensor.matmul(
                            psO[:, ht, :],
                            z_sb[64 * p:64 * p + 64,
                                 128 * ht:128 * ht + 128],
                            cw_sb[64 * p:64 * p + 64, :],
                            tile_position=(64 * p, 0))
                    nc.scalar.copy(outt[:, p], psO[:])
                nc.gpsimd.dma_start(
                    out_d[co:co + 2].rearrange("c (t h) w -> h c t w", t=2),
                    outt[:], accum_op=A.add)

    nc.compile()
    return nc


_NC_CACHE = {}


def kernel(x, W_lin, w1r, w1i, w2r, w2i):
    x = np.asarray(x)
    FH, FW, GH1, GH2, CW2 = _consts()
    wlt = np.ascontiguousarray(np.asarray(W_lin).T).astype(BF_NP)
    wmix = _wmix_slices(np.asarray(w1r), np.asarray(w1i),
                        np.asarray(w2r), np.asarray(w2i))

    if "nc" not in _NC_CACHE:
        _NC_CACHE["nc"] = _build_nc()
    nc = _NC_CACHE["nc"]

    in_maps = []
    for k in range(NCORES):
        in_maps.append({
            "x": np.ascontiguousarray(x[k]).astype(BF_NP),
            "wlt": wlt, "fh": FH, "fw": FW,
            "gh1": GH1, "gh2": GH2, "cw": CW2,
            "wmix": wmix[k],
        })
    res = run_bass_kernel_spmd(nc, in_maps, list(range(NCORES)))
    out = np.stack([res.results[k]["out"] for k in range(NCORES)], axis=0)
    return out.astype(np.float32)


# revision 7
# speedup vs baseline: 1.1869x; 1.0300x over previous
"""Distributed FNO block on 8 TRN2 NeuronCores — v2.

Strategy: batch-parallel (B=8 -> one batch element per core) for the channel
mixer and both spatial DFTs; mode-parallel (ky sharded, 4 modes per core) for
the spectral channel mixing, with an AllToAll in each direction.

v2 changes vs baseline:
  - y0 (channel mixer) writes straight to out_d; the inverse stage
    accumulates into it with SWDGE dma accum (kills the y0s roundtrip).
  - out_d is bf16 (host casts back to f32).
  - y0 compute is issued AFTER the first AllToAll so it fills the
    collective's dead window on the TensorE queue.
  - wmix is stored host-side in a DMA-contiguous layout and streamed on the
    scalar HWDGE queue, prefetched from early in the kernel.
  - forward W-DFT splits into two PSUM banks so the complex combine reads
    PSUM directly (2 DVE ops/channel instead of 4).
  - inverse processes channel pairs with tile_position packing (full
    128-partition PSUM tiles, concurrent sub-array matmuls).
  - engine rebalance: ACT evacuates PSUM, GpSimd does the mode-mix
    rearranges and collectives, sync/scalar issue all bulk DMA (HWDGE).

Self-contained: shapes/sharding hardcoded, no sibling imports.
"""
import numpy as np
import ml_dtypes
from contextlib import ExitStack

import concourse.bass as bass
import concourse.bacc as bacc
import concourse.tile as tile
from concourse import mybir
from concourse.bass_utils import run_bass_kernel_spmd

B, C, H, W = 8, 128, 256, 256
M0, M1 = 32, 32
NCORES = 8
KX = np.concatenate([np.arange(32), np.arange(H - 32, H)])  # 64 kept kx modes
BF = mybir.dt.bfloat16
F32 = mybir.dt.float32
BF_NP = ml_dtypes.bfloat16


# ----------------------------------------------------------------- host consts
def _consts():
    h = np.arange(H)[:, None]
    w = np.arange(W)[:, None]
    ky = np.arange(M1)
    th = 2 * np.pi * h * KX[None, :] / H
    FH = np.concatenate([np.cos(th), -np.sin(th)], axis=1)  # [256, 128]
    tw = 2 * np.pi * w * ky[None, :] / W
    # [cos | -sin | +sin] so Sr/Si accumulate directly in PSUM
    FW = np.concatenate([np.cos(tw), -np.sin(tw), np.sin(tw)], axis=1)  # [256,96]
    thi = 2 * np.pi * np.arange(H)[None, :] * KX[:, None] / H  # [64, 256]
    GH1 = np.concatenate([np.cos(thi) / H, -np.sin(thi) / H], axis=0)  # [128,256]
    GH2 = np.concatenate([np.sin(thi) / H, np.cos(thi) / H], axis=0)   # [128,256]
    twi = 2 * np.pi * ky[:, None] * np.arange(W)[None, :] / W  # [32, 256]
    wt = np.where(ky == 0, 1.0, 2.0)[:, None]
    CW = np.concatenate([wt * np.cos(twi) / W, -wt * np.sin(twi) / W], axis=0)
    CW[32, :] = 0.0  # irfft drops Im(Y[ky=0])
    # duplicated into both partition halves for the row-tiled inverse-W matmul
    CW2 = np.concatenate([CW, CW], axis=0)  # [128, 256]
    return (FH.astype(BF_NP), FW.astype(BF_NP), GH1.astype(BF_NP),
            GH2.astype(BF_NP), CW2.astype(BF_NP))


def _wmix_slices(w1r, w1i, w2r, w2i):
    """Per-core spectral weight slice, DMA-contiguous for the wblk tiles.

    Layout [4ky, 8kxb, ci, 8kx, 2ri, co] so one 512KB DMA fills a
    [C, 8, 2, C] SBUF tile contiguously per partition."""
    wr = np.concatenate([w1r, w2r], axis=2)  # [ci, co, 64kx, 32ky]
    wi = np.concatenate([w1i, w2i], axis=2)
    wall = np.stack([wr, wi], axis=0)  # [ri, ci, co, kx, ky]
    # -> [ky, kxb, ci, kxi, ri, co]
    wall = wall.reshape(2, C, C, 8, 8, 32)
    wall = wall.transpose(5, 3, 1, 4, 0, 2)  # ky, kxb, ci, kxi, ri, co
    wall = np.ascontiguousarray(wall).astype(BF_NP)
    return [np.ascontiguousarray(wall[4 * k:4 * k + 4]) for k in range(NCORES)]


# ----------------------------------------------------------------- bass kernel
def _build_nc():
    nc = bacc.Bacc(num_devices=NCORES)

    x_d = nc.declare_dram_parameter("x", [C, H, W], BF, isOutput=False)
    wlt_d = nc.declare_dram_parameter("wlt", [C, C], BF, isOutput=False)
    fh_d = nc.declare_dram_parameter("fh", [H, 128], BF, isOutput=False)
    fw_d = nc.declare_dram_parameter("fw", [W, 96], BF, isOutput=False)
    gh1_d = nc.declare_dram_parameter("gh1", [128, H], BF, isOutput=False)
    gh2_d = nc.declare_dram_parameter("gh2", [128, H], BF, isOutput=False)
    cw_d = nc.declare_dram_parameter("cw", [128, W], BF, isOutput=False)
    wmix_d = nc.declare_dram_parameter("wmix", [4, 8, C, 8, 2, C], BF,
                                       isOutput=False)
    out_d = nc.declare_dram_parameter("out", [C, H, W], BF, isOutput=True)

    # internal DRAM
    send1 = nc.dram_tensor("send1", [8, C, 4, 2, 64], BF)
    recv1 = nc.dram_tensor("recv1", [8, C, 4, 2, 64], BF)
    send2 = nc.dram_tensor("send2", [8, 4, C, 2, 64], BF)
    recv2 = nc.dram_tensor("recv2", [8, 4, C, 2, 64], BF)

    rg = [list(range(NCORES))]
    A = mybir.AluOpType

    with tile.TileContext(nc) as tc, ExitStack() as ctx:
        cpool = ctx.enter_context(tc.tile_pool(name="consts", bufs=1))
        spool = ctx.enter_context(tc.tile_pool(name="stages", bufs=1))
        xqpool = ctx.enter_context(tc.tile_pool(name="xq", bufs=6))
        xhpool = ctx.enter_context(tc.tile_pool(name="xh", bufs=6))
        ypool = ctx.enter_context(tc.tile_pool(name="y", bufs=3))
        wpool = ctx.enter_context(tc.tile_pool(name="wmix", bufs=6))
        zpool = ctx.enter_context(tc.tile_pool(name="z", bufs=4))
        opool = ctx.enter_context(tc.tile_pool(name="o", bufs=4))

        # constants into SBUF
        fh_sb = [cpool.tile([128, 128], BF, tag=f"fh{t}", name=f"fh{t}")
                 for t in range(2)]
        fw_sb = [cpool.tile([128, 96], BF, tag=f"fw{t}", name=f"fw{t}")
                 for t in range(2)]
        for t in range(2):
            nc.sync.dma_start(fh_sb[t][:], fh_d[128 * t:128 * (t + 1), :])
            nc.sync.dma_start(fw_sb[t][:], fw_d[128 * t:128 * (t + 1), :])
        wlt_sb = cpool.tile([C, C], BF, tag="wlt")
        nc.scalar.dma_start(wlt_sb[:], wlt_d[:])
        gh1_sb = cpool.tile([128, H], BF, tag="gh1")
        gh2_sb = cpool.tile([128, H], BF, tag="gh2")
        cw_sb = cpool.tile([128, W], BF, tag="cw")
        nc.scalar.dma_start(gh1_sb[:], gh1_d[:])
        nc.scalar.dma_start(gh2_sb[:], gh2_d[:])
        nc.scalar.dma_start(cw_sb[:], cw_d[:])

        # big staging tiles
        stage1 = spool.tile([64, C, 64], BF, tag="stage1")  # [(ri ky), c, kx]
        mm_in = spool.tile([C, 8, 4, 2, 64], BF, tag="mm_in")  # A2A#1 recv
        rhs2s = spool.tile([C, 4, 64, 2, 8], BF, tag="rhs2s")  # [-Xi | Xr]
        stage2 = spool.tile([C, 8, 4, 2, 64], BF, tag="stage2")  # modemix out
        inv2 = spool.tile([128, 8, 4, C], BF, tag="inv2")      # A2A#2 recv

        # wmix streaming: 6 blocks prefetch upfront (SWDGE, gpsimd queue),
        # the rest are issued just-in-time inside the mix loop so pool
        # recycling never stalls a queue that later phases depend on.
        wblk_tiles = [None] * 32

        def issue_wblk(i):
            kyi, kxb = divmod(i, 8)
            t = wpool.tile([C, 8, 2, C], BF, tag="wblk", name="wblk")
            nc.scalar.dma_start(t[:], wmix_d[kyi, kxb])
            wblk_tiles[i] = t

        for i in range(6):
            issue_wblk(i)

        # ---------------- forward truncated DFT (per channel) ----------------
        with tc.tile_pool(name="psA", bufs=4, space="PSUM") as psA_p, \
             tc.tile_pool(name="psW", bufs=4, space="PSUM") as psW_p:
            for blk in range(32):
                xq = xqpool.tile([128, 4, 2, 256], BF, tag="xq", name="xq")
                nc.sync.dma_start(
                    xq[:], x_d[4 * blk:4 * blk + 4].rearrange(
                        "c (t h) w -> h c t w", t=2))
                for ci in range(4):
                    c = 4 * blk + ci
                    xh = [None, None]
                    for wt_i in range(2):
                        ps = psA_p.tile([128, 128], F32, tag="psA",
                                        padded_shape=(None, 512))
                        for ht in range(2):
                            nc.tensor.matmul(
                                ps[:],
                                xq[:, ci, ht, 128 * wt_i:128 * (wt_i + 1)],
                                fh_sb[ht][:], start=(ht == 0), stop=(ht == 1))
                        xh[wt_i] = xhpool.tile([128, 128], BF, tag="xh",
                                               name="xh")
                        if wt_i == 0:
                            nc.scalar.copy(xh[wt_i][:], ps[:])
                        else:
                            nc.vector.tensor_copy(xh[wt_i][:], ps[:])
                    # Sr = cos@Xr + sin@Xi (rows 0:32), Si = cos@Xi - sin@Xr
                    # (rows 32:64), accumulated in PSUM over both w-halves.
                    psRI = psW_p.tile([64, 64], F32, tag="psRI", name="psRI",
                                      padded_shape=(None, 512))
                    for wt_i in range(2):
                        st = wt_i == 0
                        sp = wt_i == 1
                        xr = xh[wt_i][:, 0:64]
                        xi = xh[wt_i][:, 64:128]
                        fw = fw_sb[wt_i]
                        nc.tensor.matmul(psRI[0:32, :], fw[:, 0:32], xr,
                                         start=st, stop=False,
                                         tile_position=(0, 0))
                        nc.tensor.matmul(psRI[32:64, :], fw[:, 0:32], xi,
                                         start=st, stop=False,
                                         tile_position=(0, 32))
                        nc.tensor.matmul(psRI[0:32, :], fw[:, 64:96], xi,
                                         start=False, stop=sp,
                                         tile_position=(0, 0))
                        nc.tensor.matmul(psRI[32:64, :], fw[:, 32:64], xr,
                                         start=False, stop=sp,
                                         tile_position=(0, 32))
                    nc.vector.tensor_copy(stage1[:, c, :], psRI[:])

        # A2A #1: ky-shard the spectrum (c-major payload)
        for g in range(8):
            for r in range(2):
                nc.sync.dma_start(
                    send1[g][:, :, r, :].rearrange("c k x -> k c x"),
                    stage1[32 * r + 4 * g:32 * r + 4 * g + 4])
        nc.gpsimd.collective_compute(
            "AllToAll", A.bypass, replica_groups=rg,
            ins=[send1[:].opt()], outs=[recv1[:].opt()])

        # mm_in + rhs prep (gpsimd tensor ops; queue order after A2A#1)
        nc.sync.dma_start(mm_in[:], recv1[:].rearrange("b c k r x -> c b k r x"))
        # rhs2s[:, k, x, 0, b] = -Xi, rhs2s[:, k, x, 1, b] = Xr (per-kyi chunks
        # so the mix can start after the first); rhs1 is read via a strided
        # view of mm_in directly in the matmul.
        for kyi in range(4):
            nc.vector.tensor_scalar_mul(
                rhs2s[:, kyi, :, 0, :],
                mm_in[:, :, kyi, 1, :].rearrange("c b x -> c x b"), -1.0)
            nc.vector.tensor_copy(
                rhs2s[:, kyi, :, 1, :],
                mm_in[:, :, kyi, 0, :].rearrange("c b x -> c x b"))

        # ---------------- y0 = W_lin @ x -> out_d (fills A2A#1 window) -------
        with tc.tile_pool(name="psy", bufs=2, space="PSUM") as psy_p, \
             tc.tile_pool(name="psm", bufs=4, space="PSUM") as psm_p:
            def y0_iter(t):
                xt = ypool.tile([C, 16, W], BF, tag="yx", name="yx")
                nc.scalar.dma_start(xt[:], x_d[:, 16 * t:16 * t + 16, :])
                y0t = ypool.tile([C, 16, W], BF, tag="y0t", name="y0t")
                for j in range(8):
                    psy = psy_p.tile([C, 2, W], F32, tag="psy")
                    nc.tensor.matmul(psy[:], wlt_sb[:],
                                     xt[:, 2 * j:2 * j + 2, :])
                    if j % 2 == 0:
                        nc.vector.tensor_copy(y0t[:, 2 * j:2 * j + 2, :],
                                              psy[:])
                    else:
                        nc.scalar.copy(y0t[:, 2 * j:2 * j + 2, :], psy[:])
                nc.sync.dma_start(out_d[:, 16 * t:16 * t + 16, :], y0t[:])

            for t in range(12):
                y0_iter(t)

            # ---------------- modemix (ky-sharded, all batches) --------------
            for kyi in range(4):
                for kxb in range(8):
                    i = 8 * kyi + kxb
                    if i + 6 < 32:
                        issue_wblk(i + 6)
                    wblk = wblk_tiles[i]
                    wblk_tiles[i] = None
                    for half in range(2):
                        psm = psm_p.tile([C, 4, 2, 8], F32, tag="psm",
                                         padded_shape=(None, None, None, 64))
                        for kxi in range(4):
                            slot = 4 * half + kxi
                            kx = 8 * kxb + slot
                            nc.tensor.matmul(
                                psm[:, kxi], wblk[:, slot, 0, :],
                                mm_in[:, :, kyi, :, kx].rearrange(
                                    "c b r -> c r b"),
                                start=True, stop=False)
                            nc.tensor.matmul(psm[:, kxi], wblk[:, slot, 1, :],
                                             rhs2s[:, kyi, kx, :, :],
                                             start=False, stop=True)
                        kx0 = 8 * kxb + 4 * half
                        nc.vector.tensor_copy(
                            stage2[:, :, kyi, :, kx0:kx0 + 4],
                            psm[:].rearrange("c x r b -> c b r x"))

            # A2A #2: back to batch-sharded full spectrum
            for b in range(8):
                nc.sync.dma_start(
                    send2[b].rearrange("k c r x -> c k r x"), stage2[:, b])
            nc.gpsimd.collective_compute(
                "AllToAll", A.bypass, replica_groups=rg,
                ins=[send2[:].opt()], outs=[recv2[:].opt()])

            # y0 tail fills the A2A#2 window (issued before the transpose so
            # the sync queue doesn't stall behind the recv2 wait)
            for t in range(12, 16):
                y0_iter(t)
            nc.sync.dma_start_transpose(
                inv2[:], recv2[:].rearrange("g k c r x -> (g k c) (r x)"))

        # ---------------- inverse transforms, accumulate into out_d ----------
        with tc.tile_pool(name="psZ", bufs=2, space="PSUM") as psZ_p, \
             tc.tile_pool(name="psO", bufs=4, space="PSUM") as psO_p:
            for quad in range(32):
                outt = opool.tile([128, 4, 2, W], BF, tag="outt")
                for half in range(2):
                    co = 4 * quad + 2 * half
                    # 4 concurrent col-tiled matmuls:
                    # [Yr(co);Yi(co);Yr(co+1);Yi(co+1)]
                    psZ = psZ_p.tile([128, H], F32, tag="psZ",
                                     padded_shape=(None, 512))
                    nc.tensor.matmul(psZ[0:32, :], inv2[:, :, :, co],
                                     gh1_sb[:], tile_position=(0, 0))
                    nc.tensor.matmul(psZ[32:64, :], inv2[:, :, :, co],
                                     gh2_sb[:], tile_position=(0, 32))
                    nc.tensor.matmul(psZ[64:96, :], inv2[:, :, :, co + 1],
                                     gh1_sb[:], tile_position=(0, 64))
                    nc.tensor.matmul(psZ[96:128, :], inv2[:, :, :, co + 1],
                                     gh2_sb[:], tile_position=(0, 96))
                    z_sb = zpool.tile([128, H], BF, tag="z")
                    nc.vector.tensor_copy(z_sb[:], psZ[:])
                    for p in range(2):
                        psO = psO_p.tile([128, 2, W], F32, tag="psO")
                        for ht in range(2):
                            nc.tensor.matmul(
                                psO[:, ht, :],
                                z_sb[64 * p:64 * p + 64,
                                     128 * ht:128 * ht + 128],
                                cw_sb[64 * p:64 * p + 64, :],
                                tile_position=(64 * p, 0))
                        nc.scalar.copy(outt[:, 2 * half + p], psO[:])
                nc.gpsimd.dma_start(
                    out_d[4 * quad:4 * quad + 4].rearrange(
                        "c (t h) w -> h c t w", t=2),
                    outt[:], accum_op=A.add)

    nc.compile()
    return nc


_NC_CACHE = {}


def kernel(x, W_lin, w1r, w1i, w2r, w2i):
    x = np.asarray(x)
    FH, FW, GH1, GH2, CW2 = _consts()
    wlt = np.ascontiguousarray(np.asarray(W_lin).T).astype(BF_NP)
    wmix = _wmix_slices(np.asarray(w1r), np.asarray(w1i),
                        np.asarray(w2r), np.asarray(w2i))

    if "nc" not in _NC_CACHE:
        _NC_CACHE["nc"] = _build_nc()
    nc = _NC_CACHE["nc"]

    in_maps = []
    for k in range(NCORES):
        in_maps.append({
            "x": np.ascontiguousarray(x[k]).astype(BF_NP),
            "wlt": wlt, "fh": FH, "fw": FW,
            "gh1": GH1, "gh2": GH2, "cw": CW2,
            "wmix": wmix[k],
        })
    res = run_bass_kernel_spmd(nc, in_maps, list(range(NCORES)))
    out = np.stack([res.results[k]["out"] for k in range(NCORES)], axis=0)
    return out.astype(np.float32)


# revision 10
# speedup vs baseline: 1.2966x; 1.0925x over previous
"""Distributed FNO block on 8 TRN2 NeuronCores — v2.

Strategy: batch-parallel (B=8 -> one batch element per core) for the channel
mixer and both spatial DFTs; mode-parallel (ky sharded, 4 modes per core) for
the spectral channel mixing, with an AllToAll in each direction.

v2 changes vs baseline:
  - y0 (channel mixer) writes straight to out_d; the inverse stage
    accumulates into it with SWDGE dma accum (kills the y0s roundtrip).
  - out_d is bf16 (host casts back to f32).
  - y0 compute is issued AFTER the first AllToAll so it fills the
    collective's dead window on the TensorE queue.
  - wmix is stored host-side in a DMA-contiguous layout and streamed on the
    scalar HWDGE queue, prefetched from early in the kernel.
  - forward W-DFT splits into two PSUM banks so the complex combine reads
    PSUM directly (2 DVE ops/channel instead of 4).
  - inverse processes channel pairs with tile_position packing (full
    128-partition PSUM tiles, concurrent sub-array matmuls).
  - engine rebalance: ACT evacuates PSUM, GpSimd does the mode-mix
    rearranges and collectives, sync/scalar issue all bulk DMA (HWDGE).

Self-contained: shapes/sharding hardcoded, no sibling imports.
"""
import numpy as np
import ml_dtypes
from contextlib import ExitStack

import concourse.bass as bass
import concourse.bacc as bacc
import concourse.tile as tile
from concourse import mybir
from concourse.bass_utils import run_bass_kernel_spmd

B, C, H, W = 8, 128, 256, 256
M0, M1 = 32, 32
NCORES = 8
KX = np.concatenate([np.arange(32), np.arange(H - 32, H)])  # 64 kept kx modes
BF = mybir.dt.bfloat16
F32 = mybir.dt.float32
BF_NP = ml_dtypes.bfloat16


# ----------------------------------------------------------------- host consts
def _consts():
    h = np.arange(H)[:, None]
    w = np.arange(W)[:, None]
    ky = np.arange(M1)
    th = 2 * np.pi * h * KX[None, :] / H
    FH = np.concatenate([np.cos(th), -np.sin(th)], axis=1)  # [256, 128]
    tw = 2 * np.pi * w * ky[None, :] / W
    # [cos | -sin | +sin] so Sr/Si accumulate directly in PSUM
    FW = np.concatenate([np.cos(tw), -np.sin(tw), np.sin(tw)], axis=1)  # [256,96]
    thi = 2 * np.pi * np.arange(H)[None, :] * KX[:, None] / H  # [64, 256]
    GH1 = np.concatenate([np.cos(thi) / H, -np.sin(thi) / H], axis=0)  # [128,256]
    GH2 = np.concatenate([np.sin(thi) / H, np.cos(thi) / H], axis=0)   # [128,256]
    twi = 2 * np.pi * ky[:, None] * np.arange(W)[None, :] / W  # [32, 256]
    wt = np.where(ky == 0, 1.0, 2.0)[:, None]
    CW = np.concatenate([wt * np.cos(twi) / W, -wt * np.sin(twi) / W], axis=0)
    CW[32, :] = 0.0  # irfft drops Im(Y[ky=0])
    # duplicated into both partition halves for the row-tiled inverse-W matmul
    CW2 = np.concatenate([CW, CW], axis=0)  # [128, 256]
    return (FH.astype(BF_NP), FW.astype(BF_NP), GH1.astype(BF_NP),
            GH2.astype(BF_NP), CW2.astype(BF_NP))


def _wmix_slices(w1r, w1i, w2r, w2i):
    """Per-core spectral weight slice, DMA-contiguous for the wblk tiles.

    Layout [4ky, 8kxb, ci, 8kx, 2ri, co] so one 512KB DMA fills a
    [C, 8, 2, C] SBUF tile contiguously per partition."""
    wr = np.concatenate([w1r, w2r], axis=2)  # [ci, co, 64kx, 32ky]
    wi = np.concatenate([w1i, w2i], axis=2)
    wall = np.stack([wr, wi], axis=0)  # [ri, ci, co, kx, ky]
    # -> [ky, kxb, ci, kxi, ri, co]
    wall = wall.reshape(2, C, C, 8, 8, 32)
    wall = wall.transpose(5, 3, 1, 4, 0, 2)  # ky, kxb, ci, kxi, ri, co
    wall = np.ascontiguousarray(wall).astype(BF_NP)
    return [np.ascontiguousarray(wall[4 * k:4 * k + 4]) for k in range(NCORES)]


# ----------------------------------------------------------------- bass kernel
def _build_nc():
    nc = bacc.Bacc(num_devices=NCORES)

    x_d = nc.declare_dram_parameter("x", [C, H, W], BF, isOutput=False)
    wlt_d = nc.declare_dram_parameter("wlt", [C, C], BF, isOutput=False)
    fh_d = nc.declare_dram_parameter("fh", [H, 128], BF, isOutput=False)
    fw_d = nc.declare_dram_parameter("fw", [W, 96], BF, isOutput=False)
    gh1_d = nc.declare_dram_parameter("gh1", [128, H], BF, isOutput=False)
    gh2_d = nc.declare_dram_parameter("gh2", [128, H], BF, isOutput=False)
    cw_d = nc.declare_dram_parameter("cw", [128, W], BF, isOutput=False)
    wmix_d = nc.declare_dram_parameter("wmix", [4, 8, C, 8, 2, C], BF,
                                       isOutput=False)
    out_d = nc.declare_dram_parameter("out", [C, H, W], BF, isOutput=True)

    # internal DRAM
    send1a = nc.dram_tensor("send1a", [8, 64, 4, 2, 64], BF)
    recv1a = nc.dram_tensor("recv1a", [8, 64, 4, 2, 64], BF)
    send1b = nc.dram_tensor("send1b", [8, 64, 4, 2, 64], BF)
    recv1b = nc.dram_tensor("recv1b", [8, 64, 4, 2, 64], BF)
    send2 = nc.dram_tensor("send2", [8, 4, C, 2, 64], BF)
    recv2 = nc.dram_tensor("recv2", [8, 4, C, 2, 64], BF)

    rg = [list(range(NCORES))]
    A = mybir.AluOpType

    with tile.TileContext(nc) as tc, ExitStack() as ctx:
        cpool = ctx.enter_context(tc.tile_pool(name="consts", bufs=1))
        spool = ctx.enter_context(tc.tile_pool(name="stages", bufs=1))
        xqpool = ctx.enter_context(tc.tile_pool(name="xq", bufs=6))
        xhpool = ctx.enter_context(tc.tile_pool(name="xh", bufs=6))
        ypool = ctx.enter_context(tc.tile_pool(name="y", bufs=3))
        wpool = ctx.enter_context(tc.tile_pool(name="wmix", bufs=6))
        zpool = ctx.enter_context(tc.tile_pool(name="z", bufs=4))
        opool = ctx.enter_context(tc.tile_pool(name="o", bufs=4))

        # constants into SBUF
        fh_sb = [cpool.tile([128, 128], BF, tag=f"fh{t}", name=f"fh{t}")
                 for t in range(2)]
        fw_sb = [cpool.tile([128, 96], BF, tag=f"fw{t}", name=f"fw{t}")
                 for t in range(2)]
        for t in range(2):
            nc.sync.dma_start(fh_sb[t][:], fh_d[128 * t:128 * (t + 1), :])
            nc.sync.dma_start(fw_sb[t][:], fw_d[128 * t:128 * (t + 1), :])
        wlt_sb = cpool.tile([C, C], BF, tag="wlt")
        nc.scalar.dma_start(wlt_sb[:], wlt_d[:])
        gh1_sb = cpool.tile([128, H], BF, tag="gh1")
        gh2_sb = cpool.tile([128, H], BF, tag="gh2")
        cw_sb = cpool.tile([128, W], BF, tag="cw")
        nc.scalar.dma_start(gh1_sb[:], gh1_d[:])
        nc.scalar.dma_start(gh2_sb[:], gh2_d[:])
        nc.scalar.dma_start(cw_sb[:], cw_d[:])

        # big staging tiles
        stage1 = spool.tile([64, C, 64], BF, tag="stage1")  # [(ri ky), c, kx]
        mm_in = spool.tile([C, 8, 4, 2, 64], BF, tag="mm_in")  # A2A#1 recv
        rhs2s = spool.tile([C, 4, 64, 2, 8], BF, tag="rhs2s")  # [-Xi | Xr]
        stage2 = spool.tile([C, 8, 4, 2, 64], BF, tag="stage2")  # modemix out
        inv2 = spool.tile([128, 8, 4, C], BF, tag="inv2")      # A2A#2 recv

        # wmix streaming: 6 blocks prefetch upfront (SWDGE, gpsimd queue),
        # the rest are issued just-in-time inside the mix loop so pool
        # recycling never stalls a queue that later phases depend on.
        wblk_tiles = [None] * 32

        def issue_wblk(i):
            kyi, kxb = divmod(i, 8)
            t = wpool.tile([C, 8, 2, C], BF, tag="wblk", name="wblk")
            nc.scalar.dma_start(t[:], wmix_d[kyi, kxb])
            wblk_tiles[i] = t

        for i in range(6):
            issue_wblk(i)

        # PE warm-up: a dense dummy-matmul burst (~4us) flips the HAM clock
        # gate to 8/8 so the phase that follows runs at 2.4 GHz not 1.2.
        def warm_pe(pool, dep_ap, n=48):
            ps = pool.tile([64, 64], F32, tag="warm", name="warm",
                           padded_shape=(None, 512), bufs=1)
            for _ in range(n):
                nc.tensor.matmul(ps[:], dep_ap, fh_sb[0][:, 0:64],
                                 start=True, stop=True)

        # ---------------- forward truncated DFT (per channel) ----------------
        with tc.tile_pool(name="psA", bufs=4, space="PSUM") as psA_p, \
             tc.tile_pool(name="psW", bufs=3, space="PSUM") as psW_p:
            warm_pe(psW_p, fh_sb[1][:, 0:64])
            for blk in range(32):
                xq = xqpool.tile([128, 4, 2, 256], BF, tag="xq", name="xq")
                nc.sync.dma_start(
                    xq[:], x_d[4 * blk:4 * blk + 4].rearrange(
                        "c (t h) w -> h c t w", t=2))
                for ci in range(4):
                    c = 4 * blk + ci
                    xh = [None, None]
                    for wt_i in range(2):
                        ps = psA_p.tile([128, 128], F32, tag="psA",
                                        padded_shape=(None, 512))
                        for ht in range(2):
                            nc.tensor.matmul(
                                ps[:],
                                xq[:, ci, ht, 128 * wt_i:128 * (wt_i + 1)],
                                fh_sb[ht][:], start=(ht == 0), stop=(ht == 1))
                        xh[wt_i] = xhpool.tile([128, 128], BF, tag="xh",
                                               name="xh")
                        if wt_i == 0:
                            nc.scalar.copy(xh[wt_i][:], ps[:])
                        else:
                            nc.vector.tensor_copy(xh[wt_i][:], ps[:])
                    # Sr = cos@Xr + sin@Xi (rows 0:32), Si = cos@Xi - sin@Xr
                    # (rows 32:64), accumulated in PSUM over both w-halves.
                    psRI = psW_p.tile([64, 64], F32, tag="psRI", name="psRI",
                                      padded_shape=(None, 512))
                    for wt_i in range(2):
                        st = wt_i == 0
                        sp = wt_i == 1
                        xr = xh[wt_i][:, 0:64]
                        xi = xh[wt_i][:, 64:128]
                        fw = fw_sb[wt_i]
                        nc.tensor.matmul(psRI[0:32, :], fw[:, 0:32], xr,
                                         start=st, stop=False,
                                         tile_position=(0, 0))
                        nc.tensor.matmul(psRI[32:64, :], fw[:, 0:32], xi,
                                         start=st, stop=False,
                                         tile_position=(0, 32))
                        nc.tensor.matmul(psRI[0:32, :], fw[:, 64:96], xi,
                                         start=False, stop=sp,
                                         tile_position=(0, 0))
                        nc.tensor.matmul(psRI[32:64, :], fw[:, 32:64], xr,
                                         start=False, stop=sp,
                                         tile_position=(0, 32))
                    nc.vector.tensor_copy(stage1[:, c, :], psRI[:])
                if blk == 15:
                    # first 64 channels done: ship them while fwd continues
                    for g in range(8):
                        for r in range(2):
                            eng = nc.sync if g % 2 == 0 else nc.scalar
                            eng.dma_start(
                                send1a[g][:, :, r, :].rearrange(
                                    "c k x -> k c x"),
                                stage1[32 * r + 4 * g:32 * r + 4 * g + 4,
                                       0:64, :])
                    nc.gpsimd.collective_compute(
                        "AllToAll", A.bypass, replica_groups=rg,
                        ins=[send1a[:].opt()], outs=[recv1a[:].opt()])

        # A2A #1b: remaining channels
        for g in range(8):
            for r in range(2):
                eng = nc.sync if g % 2 == 0 else nc.scalar
                eng.dma_start(
                    send1b[g][:, :, r, :].rearrange("c k x -> k c x"),
                    stage1[32 * r + 4 * g:32 * r + 4 * g + 4, 64:128, :])
        nc.gpsimd.collective_compute(
            "AllToAll", A.bypass, replica_groups=rg,
            ins=[send1b[:].opt()], outs=[recv1b[:].opt()])

        nc.sync.dma_start(mm_in[0:64, :],
                          recv1a[:].rearrange("b c k r x -> c b k r x"))
        nc.sync.dma_start(mm_in[64:128, :],
                          recv1b[:].rearrange("b c k r x -> c b k r x"))
        # rhs2s[:, k, x, 0, b] = -Xi, rhs2s[:, k, x, 1, b] = Xr (per-kyi chunks
        # so the mix can start after the first); rhs1 is read via a strided
        # view of mm_in directly in the matmul.
        for kyi in range(4):
            nc.vector.tensor_scalar_mul(
                rhs2s[:, kyi, :, 0, :],
                mm_in[:, :, kyi, 1, :].rearrange("c b x -> c x b"), -1.0)
            nc.vector.tensor_copy(
                rhs2s[:, kyi, :, 1, :],
                mm_in[:, :, kyi, 0, :].rearrange("c b x -> c x b"))

        # ---------------- y0 = W_lin @ x -> out_d (fills A2A#1 window) -------
        with tc.tile_pool(name="psy", bufs=2, space="PSUM") as psy_p, \
             tc.tile_pool(name="psm", bufs=4, space="PSUM") as psm_p:
            def y0_iter(t):
                xt = ypool.tile([C, 16, W], BF, tag="yx", name="yx")
                nc.scalar.dma_start(xt[:], x_d[:, 16 * t:16 * t + 16, :])
                y0t = ypool.tile([C, 16, W], BF, tag="y0t", name="y0t")
                for j in range(8):
                    psy = psy_p.tile([C, 2, W], F32, tag="psy")
                    nc.tensor.matmul(psy[:], wlt_sb[:],
                                     xt[:, 2 * j:2 * j + 2, :])
                    if j % 2 == 0:
                        nc.vector.tensor_copy(y0t[:, 2 * j:2 * j + 2, :],
                                              psy[:])
                    else:
                        nc.scalar.copy(y0t[:, 2 * j:2 * j + 2, :], psy[:])
                nc.sync.dma_start(out_d[:, 16 * t:16 * t + 16, :], y0t[:])

            for t in range(12):
                y0_iter(t)

            # ---------------- modemix (ky-sharded, all batches) --------------
            warm_pe(psm_p, mm_in[:, 0, 0, 0, 0:64])
            for kyi in range(4):
                for kxb in range(8):
                    i = 8 * kyi + kxb
                    if i + 6 < 32:
                        issue_wblk(i + 6)
                    wblk = wblk_tiles[i]
                    wblk_tiles[i] = None
                    for half in range(2):
                        psm = psm_p.tile([C, 4, 2, 8], F32, tag="psm",
                                         padded_shape=(None, None, None, 64))
                        for kxi in range(4):
                            slot = 4 * half + kxi
                            kx = 8 * kxb + slot
                            nc.tensor.matmul(
                                psm[:, kxi], wblk[:, slot, 0, :],
                                mm_in[:, :, kyi, :, kx].rearrange(
                                    "c b r -> c r b"),
                                start=True, stop=False)
                            nc.tensor.matmul(psm[:, kxi], wblk[:, slot, 1, :],
                                             rhs2s[:, kyi, kx, :, :],
                                             start=False, stop=True)
                        kx0 = 8 * kxb + 4 * half
                        nc.vector.tensor_copy(
                            stage2[:, :, kyi, :, kx0:kx0 + 4],
                            psm[:].rearrange("c x r b -> c b r x"))

            # A2A #2: back to batch-sharded full spectrum
            for b in range(8):
                eng = nc.sync if b % 2 == 0 else nc.scalar
                eng.dma_start(
                    send2[b].rearrange("k c r x -> c k r x"), stage2[:, b])
            nc.gpsimd.collective_compute(
                "AllToAll", A.bypass, replica_groups=rg,
                ins=[send2[:].opt()], outs=[recv2[:].opt()])

            # y0 tail fills the A2A#2 window (issued before the transpose so
            # the sync queue doesn't stall behind the recv2 wait)
            for t in range(12, 16):
                y0_iter(t)
            nc.sync.dma_start_transpose(
                inv2[:], recv2[:].rearrange("g k c r x -> (g k c) (r x)"))

        # ---------------- inverse transforms, accumulate into out_d ----------
        with tc.tile_pool(name="psZ", bufs=3, space="PSUM") as psZ_p, \
             tc.tile_pool(name="psO", bufs=4, space="PSUM") as psO_p:
            warm_pe(psZ_p, inv2[:, 0, 0, 0:64])

            def do_psZ(half):
                co = 2 * half
                # 4 concurrent col-tiled matmuls:
                # [Yr(co);Yi(co);Yr(co+1);Yi(co+1)]
                psZ = psZ_p.tile([128, H], F32, tag="psZ",
                                 padded_shape=(None, 512))
                nc.tensor.matmul(psZ[0:32, :], inv2[:, :, :, co],
                                 gh1_sb[:], tile_position=(0, 0))
                nc.tensor.matmul(psZ[32:64, :], inv2[:, :, :, co],
                                 gh2_sb[:], tile_position=(0, 32))
                nc.tensor.matmul(psZ[64:96, :], inv2[:, :, :, co + 1],
                                 gh1_sb[:], tile_position=(0, 64))
                nc.tensor.matmul(psZ[96:128, :], inv2[:, :, :, co + 1],
                                 gh2_sb[:], tile_position=(0, 96))
                z_sb = zpool.tile([128, H], BF, tag="z")
                nc.vector.tensor_copy(z_sb[:], psZ[:])
                return z_sb

            def do_psO(half, z_sb, outt):
                for p in range(2):
                    psO = psO_p.tile([128, 2, W], F32, tag="psO")
                    for ht in range(2):
                        nc.tensor.matmul(
                            psO[:, ht, :],
                            z_sb[64 * p:64 * p + 64,
                                 128 * ht:128 * ht + 128],
                            cw_sb[64 * p:64 * p + 64, :],
                            tile_position=(64 * p, 0))
                    nc.scalar.copy(outt[:, 2 * (half % 2) + p], psO[:])

            outt_tiles = {}

            def get_outt(half):
                q = half // 2
                if q not in outt_tiles:
                    outt_tiles[q] = opool.tile([128, 4, 2, W], BF,
                                               tag="outt", name="outt")
                return outt_tiles[q]

            def finish_half(half, z_sb):
                do_psO(half, z_sb, get_outt(half))
                if half % 2 == 1:
                    q = half // 2
                    nc.gpsimd.dma_start(
                        out_d[4 * q:4 * q + 4].rearrange(
                            "c (t h) w -> h c t w", t=2),
                        outt_tiles.pop(q)[:], accum_op=A.add)

            z_prev = do_psZ(0)
            for half in range(1, 64):
                z_next = do_psZ(half)
                finish_half(half - 1, z_prev)
                z_prev = z_next
            finish_half(63, z_prev)

    nc.compile()
    return nc


_NC_CACHE = {}


def kernel(x, W_lin, w1r, w1i, w2r, w2i):
    x = np.asarray(x)
    FH, FW, GH1, GH2, CW2 = _consts()
    wlt = np.ascontiguousarray(np.asarray(W_lin).T).astype(BF_NP)
    wmix = _wmix_slices(np.asarray(w1r), np.asarray(w1i),
                        np.asarray(w2r), np.asarray(w2i))

    if "nc" not in _NC_CACHE:
        _NC_CACHE["nc"] = _build_nc()
    nc = _NC_CACHE["nc"]

    in_maps = []
    for k in range(NCORES):
        in_maps.append({
            "x": np.ascontiguousarray(x[k]).astype(BF_NP),
            "wlt": wlt, "fh": FH, "fw": FW,
            "gh1": GH1, "gh2": GH2, "cw": CW2,
            "wmix": wmix[k],
        })
    res = run_bass_kernel_spmd(nc, in_maps, list(range(NCORES)))
    out = np.stack([res.results[k]["out"] for k in range(NCORES)], axis=0)
    return out.astype(np.float32)


# revision 11
# speedup vs baseline: 1.3025x; 1.0045x over previous
"""Distributed FNO block on 8 TRN2 NeuronCores — v2.

Strategy: batch-parallel (B=8 -> one batch element per core) for the channel
mixer and both spatial DFTs; mode-parallel (ky sharded, 4 modes per core) for
the spectral channel mixing, with an AllToAll in each direction.

v2 changes vs baseline:
  - y0 (channel mixer) writes straight to out_d; the inverse stage
    accumulates into it with SWDGE dma accum (kills the y0s roundtrip).
  - out_d is bf16 (host casts back to f32).
  - y0 compute is issued AFTER the first AllToAll so it fills the
    collective's dead window on the TensorE queue.
  - wmix is stored host-side in a DMA-contiguous layout and streamed on the
    scalar HWDGE queue, prefetched from early in the kernel.
  - forward W-DFT splits into two PSUM banks so the complex combine reads
    PSUM directly (2 DVE ops/channel instead of 4).
  - inverse processes channel pairs with tile_position packing (full
    128-partition PSUM tiles, concurrent sub-array matmuls).
  - engine rebalance: ACT evacuates PSUM, GpSimd does the mode-mix
    rearranges and collectives, sync/scalar issue all bulk DMA (HWDGE).

Self-contained: shapes/sharding hardcoded, no sibling imports.
"""
import numpy as np
import ml_dtypes
from contextlib import ExitStack

import concourse.bass as bass
import concourse.bacc as bacc
import concourse.tile as tile
from concourse import mybir
from concourse.bass_utils import run_bass_kernel_spmd

B, C, H, W = 8, 128, 256, 256
M0, M1 = 32, 32
NCORES = 8
KX = np.concatenate([np.arange(32), np.arange(H - 32, H)])  # 64 kept kx modes
BF = mybir.dt.bfloat16
F8 = mybir.dt.float8e4
F32 = mybir.dt.float32
BF_NP = ml_dtypes.bfloat16
F8_NP = ml_dtypes.float8_e4m3fn
WSCALE = 2048.0   # wmix scaled into fp8 range
XSCALE = 0.125    # spectrum scaled into fp8 range
# net factor carried by the mix output, removed in the CW constant


# ----------------------------------------------------------------- host consts
def _consts():
    h = np.arange(H)[:, None]
    w = np.arange(W)[:, None]
    ky = np.arange(M1)
    th = 2 * np.pi * h * KX[None, :] / H
    FH = np.concatenate([np.cos(th), -np.sin(th)], axis=1)  # [256, 128]
    tw = 2 * np.pi * w * ky[None, :] / W
    # [cos | -sin | +sin] so Sr/Si accumulate directly in PSUM
    FW = np.concatenate([np.cos(tw), -np.sin(tw), np.sin(tw)], axis=1)  # [256,96]
    thi = 2 * np.pi * np.arange(H)[None, :] * KX[:, None] / H  # [64, 256]
    GH1 = np.concatenate([np.cos(thi) / H, -np.sin(thi) / H], axis=0)  # [128,256]
    GH2 = np.concatenate([np.sin(thi) / H, np.cos(thi) / H], axis=0)   # [128,256]
    twi = 2 * np.pi * ky[:, None] * np.arange(W)[None, :] / W  # [32, 256]
    wt = np.where(ky == 0, 1.0, 2.0)[:, None]
    CW = np.concatenate([wt * np.cos(twi) / W, -wt * np.sin(twi) / W], axis=0)
    CW[32, :] = 0.0  # irfft drops Im(Y[ky=0])
    CW /= (WSCALE * XSCALE)  # undo the fp8 range scaling of the mix path
    # duplicated into both partition halves for the row-tiled inverse-W matmul
    CW2 = np.concatenate([CW, CW], axis=0)  # [128, 256]
    return (FH.astype(BF_NP), FW.astype(BF_NP), GH1.astype(BF_NP),
            GH2.astype(BF_NP), CW2.astype(BF_NP))


def _wmix_slices(w1r, w1i, w2r, w2i):
    """Per-core spectral weight slice, DMA-contiguous for the wblk tiles.

    Layout [4ky, 8kxb, ci, 8kx, 2ri, co] so one 512KB DMA fills a
    [C, 8, 2, C] SBUF tile contiguously per partition."""
    wr = np.concatenate([w1r, w2r], axis=2)  # [ci, co, 64kx, 32ky]
    wi = np.concatenate([w1i, w2i], axis=2)
    wall = np.stack([wr, wi], axis=0)  # [ri, ci, co, kx, ky]
    # -> [ky, kxb, ci, kxi, ri, co]
    wall = wall.reshape(2, C, C, 8, 8, 32)
    wall = wall.transpose(5, 3, 1, 4, 0, 2)  # ky, kxb, ci, kxi, ri, co
    wall = np.ascontiguousarray(wall * WSCALE).astype(F8_NP)
    return [np.ascontiguousarray(wall[4 * k:4 * k + 4]) for k in range(NCORES)]


# ----------------------------------------------------------------- bass kernel
def _build_nc():
    nc = bacc.Bacc(num_devices=NCORES)

    x_d = nc.declare_dram_parameter("x", [C, H, W], BF, isOutput=False)
    wlt_d = nc.declare_dram_parameter("wlt", [C, C], BF, isOutput=False)
    fh_d = nc.declare_dram_parameter("fh", [H, 128], BF, isOutput=False)
    fw_d = nc.declare_dram_parameter("fw", [W, 96], BF, isOutput=False)
    gh1_d = nc.declare_dram_parameter("gh1", [128, H], BF, isOutput=False)
    gh2_d = nc.declare_dram_parameter("gh2", [128, H], BF, isOutput=False)
    cw_d = nc.declare_dram_parameter("cw", [128, W], BF, isOutput=False)
    wmix_d = nc.declare_dram_parameter("wmix", [4, 8, C, 8, 2, C], F8,
                                       isOutput=False)
    out_d = nc.declare_dram_parameter("out", [C, H, W], BF, isOutput=True)

    # internal DRAM
    send1a = nc.dram_tensor("send1a", [8, 64, 4, 2, 64], F8)
    recv1a = nc.dram_tensor("recv1a", [8, 64, 4, 2, 64], F8)
    send1b = nc.dram_tensor("send1b", [8, 64, 4, 2, 64], F8)
    recv1b = nc.dram_tensor("recv1b", [8, 64, 4, 2, 64], F8)
    send2 = nc.dram_tensor("send2", [8, 4, C, 2, 64], BF)
    recv2 = nc.dram_tensor("recv2", [8, 4, C, 2, 64], BF)

    rg = [list(range(NCORES))]
    A = mybir.AluOpType

    with tile.TileContext(nc) as tc, ExitStack() as ctx:
        cpool = ctx.enter_context(tc.tile_pool(name="consts", bufs=1))
        spool = ctx.enter_context(tc.tile_pool(name="stages", bufs=1))
        xqpool = ctx.enter_context(tc.tile_pool(name="xq", bufs=6))
        xhpool = ctx.enter_context(tc.tile_pool(name="xh", bufs=6))
        ypool = ctx.enter_context(tc.tile_pool(name="y", bufs=3))
        wpool = ctx.enter_context(tc.tile_pool(name="wmix", bufs=6))
        zpool = ctx.enter_context(tc.tile_pool(name="z", bufs=4))
        opool = ctx.enter_context(tc.tile_pool(name="o", bufs=4))

        # constants into SBUF
        fh_sb = [cpool.tile([128, 128], BF, tag=f"fh{t}", name=f"fh{t}")
                 for t in range(2)]
        fw_sb = [cpool.tile([128, 96], BF, tag=f"fw{t}", name=f"fw{t}")
                 for t in range(2)]
        for t in range(2):
            nc.sync.dma_start(fh_sb[t][:], fh_d[128 * t:128 * (t + 1), :])
            nc.sync.dma_start(fw_sb[t][:], fw_d[128 * t:128 * (t + 1), :])
        wlt_sb = cpool.tile([C, C], BF, tag="wlt")
        nc.scalar.dma_start(wlt_sb[:], wlt_d[:])
        gh1_sb = cpool.tile([128, H], BF, tag="gh1")
        gh2_sb = cpool.tile([128, H], BF, tag="gh2")
        cw_sb = cpool.tile([128, W], BF, tag="cw")
        nc.scalar.dma_start(gh1_sb[:], gh1_d[:])
        nc.scalar.dma_start(gh2_sb[:], gh2_d[:])
        nc.scalar.dma_start(cw_sb[:], cw_d[:])

        # big staging tiles
        stage1 = spool.tile([64, C, 64], F8, tag="stage1")  # [(ri ky), c, kx]
        mm_in = spool.tile([C, 8, 4, 2, 64], F8, tag="mm_in")  # A2A#1 recv
        rhs2s = spool.tile([C, 4, 64, 2, 8], F8, tag="rhs2s")  # [-Xi | Xr]
        stage2 = spool.tile([C, 8, 4, 2, 64], BF, tag="stage2")  # modemix out
        inv2 = spool.tile([128, 8, 4, C], BF, tag="inv2")      # A2A#2 recv

        # wmix streaming: 6 blocks prefetch upfront (SWDGE, gpsimd queue),
        # the rest are issued just-in-time inside the mix loop so pool
        # recycling never stalls a queue that later phases depend on.
        wblk_tiles = [None] * 32

        def issue_wblk(i):
            kyi, kxb = divmod(i, 8)
            t = wpool.tile([C, 8, 2, C], F8, tag="wblk", name="wblk")
            nc.scalar.dma_start(t[:], wmix_d[kyi, kxb])
            wblk_tiles[i] = t

        for i in range(6):
            issue_wblk(i)

        # PE warm-up: a dense dummy-matmul burst (~4us) flips the HAM clock
        # gate to 8/8 so the phase that follows runs at 2.4 GHz not 1.2.
        def warm_pe(pool, dep_ap, n=48):
            ps = pool.tile([64, 64], F32, tag="warm", name="warm",
                           padded_shape=(None, 512), bufs=1)
            for _ in range(n):
                nc.tensor.matmul(ps[:], dep_ap, fh_sb[0][:, 0:64],
                                 start=True, stop=True)

        # ---------------- forward truncated DFT (per channel) ----------------
        with tc.tile_pool(name="psA", bufs=4, space="PSUM") as psA_p, \
             tc.tile_pool(name="psW", bufs=3, space="PSUM") as psW_p:
            warm_pe(psW_p, fh_sb[1][:, 0:64])
            for blk in range(32):
                xq = xqpool.tile([128, 4, 2, 256], BF, tag="xq", name="xq")
                nc.sync.dma_start(
                    xq[:], x_d[4 * blk:4 * blk + 4].rearrange(
                        "c (t h) w -> h c t w", t=2))
                for ci in range(4):
                    c = 4 * blk + ci
                    xh = [None, None]
                    for wt_i in range(2):
                        ps = psA_p.tile([128, 128], F32, tag="psA",
                                        padded_shape=(None, 512))
                        for ht in range(2):
                            nc.tensor.matmul(
                                ps[:],
                                xq[:, ci, ht, 128 * wt_i:128 * (wt_i + 1)],
                                fh_sb[ht][:], start=(ht == 0), stop=(ht == 1))
                        xh[wt_i] = xhpool.tile([128, 128], BF, tag="xh",
                                               name="xh")
                        if wt_i == 0:
                            nc.scalar.copy(xh[wt_i][:], ps[:])
                        else:
                            nc.vector.tensor_copy(xh[wt_i][:], ps[:])
                    # Sr = cos@Xr + sin@Xi (rows 0:32), Si = cos@Xi - sin@Xr
                    # (rows 32:64), accumulated in PSUM over both w-halves.
                    psRI = psW_p.tile([64, 64], F32, tag="psRI", name="psRI",
                                      padded_shape=(None, 512))
                    for wt_i in range(2):
                        st = wt_i == 0
                        sp = wt_i == 1
                        xr = xh[wt_i][:, 0:64]
                        xi = xh[wt_i][:, 64:128]
                        fw = fw_sb[wt_i]
                        nc.tensor.matmul(psRI[0:32, :], fw[:, 0:32], xr,
                                         start=st, stop=False,
                                         tile_position=(0, 0))
                        nc.tensor.matmul(psRI[32:64, :], fw[:, 0:32], xi,
                                         start=st, stop=False,
                                         tile_position=(0, 32))
                        nc.tensor.matmul(psRI[0:32, :], fw[:, 64:96], xi,
                                         start=False, stop=sp,
                                         tile_position=(0, 0))
                        nc.tensor.matmul(psRI[32:64, :], fw[:, 32:64], xr,
                                         start=False, stop=sp,
                                         tile_position=(0, 32))
                    nc.vector.tensor_scalar_mul(stage1[:, c, :],
                                                psRI[:], XSCALE)
                if blk == 15:
                    # first 64 channels done: ship them while fwd continues
                    for g in range(8):
                        for r in range(2):
                            eng = nc.sync if g % 2 == 0 else nc.scalar
                            eng.dma_start(
                                send1a[g][:, :, r, :].rearrange(
                                    "c k x -> k c x"),
                                stage1[32 * r + 4 * g:32 * r + 4 * g + 4,
                                       0:64, :])
                    nc.gpsimd.collective_compute(
                        "AllToAll", A.bypass, replica_groups=rg,
                        ins=[send1a[:].opt()], outs=[recv1a[:].opt()])

        # A2A #1b: remaining channels
        for g in range(8):
            for r in range(2):
                eng = nc.sync if g % 2 == 0 else nc.scalar
                eng.dma_start(
                    send1b[g][:, :, r, :].rearrange("c k x -> k c x"),
                    stage1[32 * r + 4 * g:32 * r + 4 * g + 4, 64:128, :])
        nc.gpsimd.collective_compute(
            "AllToAll", A.bypass, replica_groups=rg,
            ins=[send1b[:].opt()], outs=[recv1b[:].opt()])

        nc.sync.dma_start(mm_in[0:64, :],
                          recv1a[:].rearrange("b c k r x -> c b k r x"))
        nc.sync.dma_start(mm_in[64:128, :],
                          recv1b[:].rearrange("b c k r x -> c b k r x"))
        # rhs2s[:, k, x, 0, b] = -Xi, rhs2s[:, k, x, 1, b] = Xr (per-kyi chunks
        # so the mix can start after the first); rhs1 is read via a strided
        # view of mm_in directly in the matmul.
        for kyi in range(4):
            nc.vector.tensor_scalar_mul(
                rhs2s[:, kyi, :, 0, :],
                mm_in[:, :, kyi, 1, :].rearrange("c b x -> c x b"), -1.0)
            nc.vector.tensor_copy(
                rhs2s[:, kyi, :, 1, :],
                mm_in[:, :, kyi, 0, :].rearrange("c b x -> c x b"))

        # ---------------- y0 = W_lin @ x -> out_d (fills A2A#1 window) -------
        with tc.tile_pool(name="psy", bufs=2, space="PSUM") as psy_p, \
             tc.tile_pool(name="psm", bufs=4, space="PSUM") as psm_p:
            def y0_iter(t):
                xt = ypool.tile([C, 16, W], BF, tag="yx", name="yx")
                nc.scalar.dma_start(xt[:], x_d[:, 16 * t:16 * t + 16, :])
                y0t = ypool.tile([C, 16, W], BF, tag="y0t", name="y0t")
                for j in range(8):
                    psy = psy_p.tile([C, 2, W], F32, tag="psy")
                    nc.tensor.matmul(psy[:], wlt_sb[:],
                                     xt[:, 2 * j:2 * j + 2, :])
                    if j % 2 == 0:
                        nc.vector.tensor_copy(y0t[:, 2 * j:2 * j + 2, :],
                                              psy[:])
                    else:
                        nc.scalar.copy(y0t[:, 2 * j:2 * j + 2, :], psy[:])
                nc.sync.dma_start(out_d[:, 16 * t:16 * t + 16, :], y0t[:])

            for t in range(12):
                y0_iter(t)

            # ---------------- modemix (ky-sharded, all batches) --------------
            warm_pe(psm_p, mm_in[:, 0, 0, 0, 0:64])
            for kyi in range(4):
                if kyi > 0:
                    warm_pe(psm_p, mm_in[:, 0, 0, 0, 0:64], n=20)
                for kxb in range(8):
                    i = 8 * kyi + kxb
                    if i + 6 < 32:
                        issue_wblk(i + 6)
                    wblk = wblk_tiles[i]
                    wblk_tiles[i] = None
                    for half in range(2):
                        psm = psm_p.tile([C, 4, 2, 8], F32, tag="psm",
                                         padded_shape=(None, None, None, 64))
                        for kxi in range(4):
                            slot = 4 * half + kxi
                            kx = 8 * kxb + slot
                            nc.tensor.matmul(
                                psm[:, kxi], wblk[:, slot, 0, :],
                                mm_in[:, :, kyi, :, kx].rearrange(
                                    "c b r -> c r b"),
                                start=True, stop=False)
                            nc.tensor.matmul(psm[:, kxi], wblk[:, slot, 1, :],
                                             rhs2s[:, kyi, kx, :, :],
                                             start=False, stop=True)
                        kx0 = 8 * kxb + 4 * half
                        nc.vector.tensor_copy(
                            stage2[:, :, kyi, :, kx0:kx0 + 4],
                            psm[:].rearrange("c x r b -> c b r x"))

            # A2A #2: back to batch-sharded full spectrum
            for b in range(8):
                eng = nc.sync if b % 2 == 0 else nc.scalar
                eng.dma_start(
                    send2[b].rearrange("k c r x -> c k r x"), stage2[:, b])
            nc.gpsimd.collective_compute(
                "AllToAll", A.bypass, replica_groups=rg,
                ins=[send2[:].opt()], outs=[recv2[:].opt()])

            # y0 tail fills the A2A#2 window (issued before the transpose so
            # the sync queue doesn't stall behind the recv2 wait)
            for t in range(12, 16):
                y0_iter(t)
            nc.sync.dma_start_transpose(
                inv2[:], recv2[:].rearrange("g k c r x -> (g k c) (r x)"))

        # ---------------- inverse transforms, accumulate into out_d ----------
        with tc.tile_pool(name="psZ", bufs=3, space="PSUM") as psZ_p, \
             tc.tile_pool(name="psO", bufs=4, space="PSUM") as psO_p:
            warm_pe(psZ_p, inv2[:, 0, 0, 0:64])

            def do_psZ(half):
                co = 2 * half
                # 4 concurrent col-tiled matmuls:
                # [Yr(co);Yi(co);Yr(co+1);Yi(co+1)]
                psZ = psZ_p.tile([128, H], F32, tag="psZ",
                                 padded_shape=(None, 512))
                nc.tensor.matmul(psZ[0:32, :], inv2[:, :, :, co],
                                 gh1_sb[:], tile_position=(0, 0))
                nc.tensor.matmul(psZ[32:64, :], inv2[:, :, :, co],
                                 gh2_sb[:], tile_position=(0, 32))
                nc.tensor.matmul(psZ[64:96, :], inv2[:, :, :, co + 1],
                                 gh1_sb[:], tile_position=(0, 64))
                nc.tensor.matmul(psZ[96:128, :], inv2[:, :, :, co + 1],
                                 gh2_sb[:], tile_position=(0, 96))
                z_sb = zpool.tile([128, H], BF, tag="z")
                nc.vector.tensor_copy(z_sb[:], psZ[:])
                return z_sb

            def do_psO(half, z_sb, outt):
                for p in range(2):
                    psO = psO_p.tile([128, 2, W], F32, tag="psO")
                    for ht in range(2):
                        nc.tensor.matmul(
                            psO[:, ht, :],
                            z_sb[64 * p:64 * p + 64,
                                 128 * ht:128 * ht + 128],
                            cw_sb[64 * p:64 * p + 64, :],
                            tile_position=(64 * p, 0))
                    nc.scalar.copy(outt[:, 2 * (half % 2) + p], psO[:])

            outt_tiles = {}

            def get_outt(half):
                q = half // 2
                if q not in outt_tiles:
                    outt_tiles[q] = opool.tile([128, 4, 2, W], BF,
                                               tag="outt", name="outt")
                return outt_tiles[q]

            def finish_half(half, z_sb):
                do_psO(half, z_sb, get_outt(half))
                if half % 2 == 1:
                    q = half // 2
                    nc.gpsimd.dma_start(
                        out_d[4 * q:4 * q + 4].rearrange(
                            "c (t h) w -> h c t w", t=2),
                        outt_tiles.pop(q)[:], accum_op=A.add)

            z_prev = do_psZ(0)
            for half in range(1, 64):
                if half % 12 == 0:
                    warm_pe(psZ_p, inv2[:, 0, 0, 0:64], n=20)
                z_next = do_psZ(half)
                finish_half(half - 1, z_prev)
                z_prev = z_next
            finish_half(63, z_prev)

    nc.compile()
    return nc


_NC_CACHE = {}


def kernel(x, W_lin, w1r, w1i, w2r, w2i):
    x = np.asarray(x)
    FH, FW, GH1, GH2, CW2 = _consts()
    wlt = np.ascontiguousarray(np.asarray(W_lin).T).astype(BF_NP)
    wmix = _wmix_slices(np.asarray(w1r), np.asarray(w1i),
                        np.asarray(w2r), np.asarray(w2i))

    if "nc" not in _NC_CACHE:
        _NC_CACHE["nc"] = _build_nc()
    nc = _NC_CACHE["nc"]

    in_maps = []
    for k in range(NCORES):
        in_maps.append({
            "x": np.ascontiguousarray(x[k]).astype(BF_NP),
            "wlt": wlt, "fh": FH, "fw": FW,
            "gh1": GH1, "gh2": GH2, "cw": CW2,
            "wmix": wmix[k],
        })
    res = run_bass_kernel_spmd(nc, in_maps, list(range(NCORES)))
    out = np.stack([res.results[k]["out"] for k in range(NCORES)], axis=0)
    return out.astype(np.float32)


# revision 13
# speedup vs baseline: 1.3250x; 1.0173x over previous
"""Distributed FNO block on 8 TRN2 NeuronCores — v2.

Strategy: batch-parallel (B=8 -> one batch element per core) for the channel
mixer and both spatial DFTs; mode-parallel (ky sharded, 4 modes per core) for
the spectral channel mixing, with an AllToAll in each direction.

v2 changes vs baseline:
  - y0 (channel mixer) writes straight to out_d; the inverse stage
    accumulates into it with SWDGE dma accum (kills the y0s roundtrip).
  - out_d is bf16 (host casts back to f32).
  - y0 compute is issued AFTER the first AllToAll so it fills the
    collective's dead window on the TensorE queue.
  - wmix is stored host-side in a DMA-contiguous layout and streamed on the
    scalar HWDGE queue, prefetched from early in the kernel.
  - forward W-DFT splits into two PSUM banks so the complex combine reads
    PSUM directly (2 DVE ops/channel instead of 4).
  - inverse processes channel pairs with tile_position packing (full
    128-partition PSUM tiles, concurrent sub-array matmuls).
  - engine rebalance: ACT evacuates PSUM, GpSimd does the mode-mix
    rearranges and collectives, sync/scalar issue all bulk DMA (HWDGE).

Self-contained: shapes/sharding hardcoded, no sibling imports.
"""
import numpy as np
import ml_dtypes
from contextlib import ExitStack

import concourse.bass as bass
import concourse.bacc as bacc
import concourse.tile as tile
from concourse import mybir
from concourse.bass_utils import run_bass_kernel_spmd

B, C, H, W = 8, 128, 256, 256
M0, M1 = 32, 32
NCORES = 8
KX = np.concatenate([np.arange(32), np.arange(H - 32, H)])  # 64 kept kx modes
BF = mybir.dt.bfloat16
F8 = mybir.dt.float8e4
F32 = mybir.dt.float32
BF_NP = ml_dtypes.bfloat16
F8_NP = ml_dtypes.float8_e4m3fn
WSCALE = 2048.0   # wmix scaled into fp8 range
XSCALE = 0.125    # spectrum scaled into fp8 range
# net factor carried by the mix output, removed in the CW constant


# ----------------------------------------------------------------- host consts
def _consts():
    h = np.arange(H)[:, None]
    w = np.arange(W)[:, None]
    ky = np.arange(M1)
    th = 2 * np.pi * h * KX[None, :] / H
    FH = np.concatenate([np.cos(th), -np.sin(th)], axis=1)  # [256, 128]
    tw = 2 * np.pi * w * ky[None, :] / W
    # [cos | -sin | +sin] so Sr/Si accumulate directly in PSUM
    FW = np.concatenate([np.cos(tw), -np.sin(tw), np.sin(tw)], axis=1)  # [256,96]
    thi = 2 * np.pi * np.arange(H)[None, :] * KX[:, None] / H  # [64, 256]
    GH1 = np.concatenate([np.cos(thi) / H, -np.sin(thi) / H], axis=0)  # [128,256]
    GH2 = np.concatenate([np.sin(thi) / H, np.cos(thi) / H], axis=0)   # [128,256]
    twi = 2 * np.pi * ky[:, None] * np.arange(W)[None, :] / W  # [32, 256]
    wt = np.where(ky == 0, 1.0, 2.0)[:, None]
    CW = np.concatenate([wt * np.cos(twi) / W, -wt * np.sin(twi) / W], axis=0)
    CW[32, :] = 0.0  # irfft drops Im(Y[ky=0])
    CW /= (WSCALE * XSCALE)  # undo the fp8 range scaling of the mix path
    # duplicated into both partition halves for the row-tiled inverse-W matmul
    CW2 = np.concatenate([CW, CW], axis=0)  # [128, 256]
    return (FH.astype(BF_NP), FW.astype(BF_NP), GH1.astype(BF_NP),
            GH2.astype(BF_NP), CW2.astype(BF_NP))


def _wmix_slices(w1r, w1i, w2r, w2i):
    """Per-core spectral weight slice, DMA-contiguous for the wblk tiles.

    Layout [4ky, 8kxb, ci, 8kx, 2ri, co] so one 512KB DMA fills a
    [C, 8, 2, C] SBUF tile contiguously per partition."""
    wr = np.concatenate([w1r, w2r], axis=2)  # [ci, co, 64kx, 32ky]
    wi = np.concatenate([w1i, w2i], axis=2)
    wall = np.stack([wr, wi], axis=0)  # [ri, ci, co, kx, ky]
    # -> [ky, kxb, ci, kxi, ri, co]
    wall = wall.reshape(2, C, C, 8, 8, 32)
    wall = wall.transpose(5, 3, 1, 4, 0, 2)  # ky, kxb, ci, kxi, ri, co
    wall = np.ascontiguousarray(wall * WSCALE).astype(F8_NP)
    return [np.ascontiguousarray(wall[4 * k:4 * k + 4]) for k in range(NCORES)]


# ----------------------------------------------------------------- bass kernel
def _build_nc():
    nc = bacc.Bacc(num_devices=NCORES)

    x_d = nc.declare_dram_parameter("x", [C, H, W], BF, isOutput=False)
    wlt_d = nc.declare_dram_parameter("wlt", [C, C], BF, isOutput=False)
    fh_d = nc.declare_dram_parameter("fh", [H, 128], BF, isOutput=False)
    fw_d = nc.declare_dram_parameter("fw", [W, 96], BF, isOutput=False)
    gh1_d = nc.declare_dram_parameter("gh1", [128, H], BF, isOutput=False)
    gh2_d = nc.declare_dram_parameter("gh2", [128, H], BF, isOutput=False)
    cw_d = nc.declare_dram_parameter("cw", [128, W], BF, isOutput=False)
    wmix_d = nc.declare_dram_parameter("wmix", [4, 8, C, 8, 2, C], F8,
                                       isOutput=False)
    out_d = nc.declare_dram_parameter("out", [C, H, W], BF, isOutput=True)

    # internal DRAM
    y0s = nc.dram_tensor("y0s", [C, H, W], BF)
    send1a = nc.dram_tensor("send1a", [8, 64, 4, 2, 64], F8)
    recv1a = nc.dram_tensor("recv1a", [8, 64, 4, 2, 64], F8)
    send1b = nc.dram_tensor("send1b", [8, 64, 4, 2, 64], F8)
    recv1b = nc.dram_tensor("recv1b", [8, 64, 4, 2, 64], F8)
    send2 = nc.dram_tensor("send2", [8, 4, C, 2, 64], BF)
    recv2 = nc.dram_tensor("recv2", [8, 4, C, 2, 64], BF)

    rg = [list(range(NCORES))]
    A = mybir.AluOpType

    with tile.TileContext(nc) as tc, ExitStack() as ctx:
        cpool = ctx.enter_context(tc.tile_pool(name="consts", bufs=1))
        spool = ctx.enter_context(tc.tile_pool(name="stages", bufs=1))
        xqpool = ctx.enter_context(tc.tile_pool(name="xq", bufs=6))
        xhpool = ctx.enter_context(tc.tile_pool(name="xh", bufs=6))
        ypool = ctx.enter_context(tc.tile_pool(name="y", bufs=3))
        wpool = ctx.enter_context(tc.tile_pool(name="wmix", bufs=6))
        zpool = ctx.enter_context(tc.tile_pool(name="z", bufs=4))
        opool = ctx.enter_context(tc.tile_pool(name="o", bufs=4))

        # constants into SBUF
        fh_sb = [cpool.tile([128, 128], BF, tag=f"fh{t}", name=f"fh{t}")
                 for t in range(2)]
        fw_sb = [cpool.tile([128, 96], BF, tag=f"fw{t}", name=f"fw{t}")
                 for t in range(2)]
        for t in range(2):
            nc.sync.dma_start(fh_sb[t][:], fh_d[128 * t:128 * (t + 1), :])
            nc.sync.dma_start(fw_sb[t][:], fw_d[128 * t:128 * (t + 1), :])
        wlt_sb = cpool.tile([C, C], BF, tag="wlt")
        nc.scalar.dma_start(wlt_sb[:], wlt_d[:])
        gh1_sb = cpool.tile([128, H], BF, tag="gh1")
        gh2_sb = cpool.tile([128, H], BF, tag="gh2")
        cw_sb = cpool.tile([128, W], BF, tag="cw")
        nc.scalar.dma_start(gh1_sb[:], gh1_d[:])
        nc.scalar.dma_start(gh2_sb[:], gh2_d[:])
        nc.scalar.dma_start(cw_sb[:], cw_d[:])

        # big staging tiles
        stage1 = spool.tile([64, C, 64], F8, tag="stage1")  # [(ri ky), c, kx]
        mm_in = spool.tile([C, 8, 4, 2, 64], F8, tag="mm_in")  # A2A#1 recv
        rhs2s = spool.tile([C, 4, 64, 2, 8], F8, tag="rhs2s")  # [-Xi | Xr]
        stage2 = spool.tile([C, 8, 4, 2, 64], BF, tag="stage2")  # modemix out
        inv2 = spool.tile([128, 8, 4, C], BF, tag="inv2")      # A2A#2 recv

        # wmix streaming: 6 blocks prefetch upfront (SWDGE, gpsimd queue),
        # the rest are issued just-in-time inside the mix loop so pool
        # recycling never stalls a queue that later phases depend on.
        wblk_tiles = [None] * 32

        def issue_wblk(i):
            kyi, kxb = divmod(i, 8)
            t = wpool.tile([C, 8, 2, C], F8, tag="wblk", name="wblk")
            nc.scalar.dma_start(t[:], wmix_d[kyi, kxb])
            wblk_tiles[i] = t

        for i in range(6):
            issue_wblk(i)

        # PE warm-up: a dense dummy-matmul burst (~4us) flips the HAM clock
        # gate to 8/8 so the phase that follows runs at 2.4 GHz not 1.2.
        def warm_pe(pool, dep_ap, n=48):
            ps = pool.tile([64, 64], F32, tag="warm", name="warm",
                           padded_shape=(None, 512), bufs=1)
            for _ in range(n):
                nc.tensor.matmul(ps[:], dep_ap, fh_sb[0][:, 0:64],
                                 start=True, stop=True)

        # ---------------- forward truncated DFT (per channel) ----------------
        with tc.tile_pool(name="psA", bufs=4, space="PSUM") as psA_p, \
             tc.tile_pool(name="psW", bufs=3, space="PSUM") as psW_p:
            warm_pe(psW_p, fh_sb[1][:, 0:64])
            for blk in range(32):
                xq = xqpool.tile([128, 4, 2, 256], BF, tag="xq", name="xq")
                nc.sync.dma_start(
                    xq[:], x_d[4 * blk:4 * blk + 4].rearrange(
                        "c (t h) w -> h c t w", t=2))
                for ci in range(4):
                    c = 4 * blk + ci
                    xh = [None, None]
                    for wt_i in range(2):
                        ps = psA_p.tile([128, 128], F32, tag="psA",
                                        padded_shape=(None, 512))
                        for ht in range(2):
                            nc.tensor.matmul(
                                ps[:],
                                xq[:, ci, ht, 128 * wt_i:128 * (wt_i + 1)],
                                fh_sb[ht][:], start=(ht == 0), stop=(ht == 1))
                        xh[wt_i] = xhpool.tile([128, 128], BF, tag="xh",
                                               name="xh")
                        if wt_i == 0:
                            nc.scalar.copy(xh[wt_i][:], ps[:])
                        else:
                            nc.vector.tensor_copy(xh[wt_i][:], ps[:])
                    # Sr = cos@Xr + sin@Xi (rows 0:32), Si = cos@Xi - sin@Xr
                    # (rows 32:64), accumulated in PSUM over both w-halves.
                    psRI = psW_p.tile([64, 64], F32, tag="psRI", name="psRI",
                                      padded_shape=(None, 512))
                    for wt_i in range(2):
                        st = wt_i == 0
                        sp = wt_i == 1
                        xr = xh[wt_i][:, 0:64]
                        xi = xh[wt_i][:, 64:128]
                        fw = fw_sb[wt_i]
                        nc.tensor.matmul(psRI[0:32, :], fw[:, 0:32], xr,
                                         start=st, stop=False,
                                         tile_position=(0, 0))
                        nc.tensor.matmul(psRI[32:64, :], fw[:, 0:32], xi,
                                         start=st, stop=False,
                                         tile_position=(0, 32))
                        nc.tensor.matmul(psRI[0:32, :], fw[:, 64:96], xi,
                                         start=False, stop=sp,
                                         tile_position=(0, 0))
                        nc.tensor.matmul(psRI[32:64, :], fw[:, 32:64], xr,
                                         start=False, stop=sp,
                                         tile_position=(0, 32))
                    nc.vector.tensor_scalar_mul(stage1[:, c, :],
                                                psRI[:], XSCALE)
                if blk == 15:
                    # first 64 channels done: ship them while fwd continues
                    for g in range(8):
                        for r in range(2):
                            eng = nc.sync if g % 2 == 0 else nc.scalar
                            eng.dma_start(
                                send1a[g][:, :, r, :].rearrange(
                                    "c k x -> k c x"),
                                stage1[32 * r + 4 * g:32 * r + 4 * g + 4,
                                       0:64, :])
                    nc.gpsimd.collective_compute(
                        "AllToAll", A.bypass, replica_groups=rg,
                        ins=[send1a[:].opt()], outs=[recv1a[:].opt()])

        # A2A #1b: remaining channels
        for g in range(8):
            for r in range(2):
                eng = nc.sync if g % 2 == 0 else nc.scalar
                eng.dma_start(
                    send1b[g][:, :, r, :].rearrange("c k x -> k c x"),
                    stage1[32 * r + 4 * g:32 * r + 4 * g + 4, 64:128, :])
        nc.gpsimd.collective_compute(
            "AllToAll", A.bypass, replica_groups=rg,
            ins=[send1b[:].opt()], outs=[recv1b[:].opt()])

        nc.sync.dma_start(mm_in[0:64, :],
                          recv1a[:].rearrange("b c k r x -> c b k r x"))
        nc.sync.dma_start(mm_in[64:128, :],
                          recv1b[:].rearrange("b c k r x -> c b k r x"))
        # rhs2s[:, k, x, 0, b] = -Xi, rhs2s[:, k, x, 1, b] = Xr (per-kyi chunks
        # so the mix can start after the first); rhs1 is read via a strided
        # view of mm_in directly in the matmul.
        for kyi in range(4):
            nc.vector.tensor_scalar_mul(
                rhs2s[:, kyi, :, 0, :],
                mm_in[:, :, kyi, 1, :].rearrange("c b x -> c x b"), -1.0)
            nc.vector.tensor_copy(
                rhs2s[:, kyi, :, 1, :],
                mm_in[:, :, kyi, 0, :].rearrange("c b x -> c x b"))

        # ---------------- y0 = W_lin @ x -> out_d (fills A2A#1 window) -------
        with tc.tile_pool(name="psy", bufs=2, space="PSUM") as psy_p, \
             tc.tile_pool(name="psm", bufs=4, space="PSUM") as psm_p:
            def y0_iter(t):
                xt = ypool.tile([C, 16, W], BF, tag="yx", name="yx")
                nc.scalar.dma_start(xt[:], x_d[:, 16 * t:16 * t + 16, :])
                y0t = ypool.tile([C, 16, W], BF, tag="y0t", name="y0t")
                for j in range(8):
                    psy = psy_p.tile([C, 2, W], F32, tag="psy")
                    nc.tensor.matmul(psy[:], wlt_sb[:],
                                     xt[:, 2 * j:2 * j + 2, :])
                    if j % 2 == 0:
                        nc.vector.tensor_copy(y0t[:, 2 * j:2 * j + 2, :],
                                              psy[:])
                    else:
                        nc.scalar.copy(y0t[:, 2 * j:2 * j + 2, :], psy[:])
                nc.sync.dma_start(y0s[:, 16 * t:16 * t + 16, :], y0t[:])

            for t in range(12):
                y0_iter(t)

            # ---------------- modemix (ky-sharded, all batches) --------------
            warm_pe(psm_p, mm_in[:, 0, 0, 0, 0:64])
            for kyi in range(4):
                if kyi > 0:
                    warm_pe(psm_p, mm_in[:, 0, 0, 0, 0:64], n=20)
                for kxb in range(8):
                    i = 8 * kyi + kxb
                    if i + 6 < 32:
                        issue_wblk(i + 6)
                    wblk = wblk_tiles[i]
                    wblk_tiles[i] = None
                    for half in range(2):
                        psm = psm_p.tile([C, 4, 2, 8], F32, tag="psm",
                                         padded_shape=(None, None, None, 64))
                        for kxi in range(4):
                            slot = 4 * half + kxi
                            kx = 8 * kxb + slot
                            nc.tensor.matmul(
                                psm[:, kxi], wblk[:, slot, 0, :],
                                mm_in[:, :, kyi, :, kx].rearrange(
                                    "c b r -> c r b"),
                                start=True, stop=False)
                            nc.tensor.matmul(psm[:, kxi], wblk[:, slot, 1, :],
                                             rhs2s[:, kyi, kx, :, :],
                                             start=False, stop=True)
                        kx0 = 8 * kxb + 4 * half
                        nc.vector.tensor_copy(
                            stage2[:, :, kyi, :, kx0:kx0 + 4],
                            psm[:].rearrange("c x r b -> c b r x"))

            # A2A #2: back to batch-sharded full spectrum
            for b in range(8):
                eng = nc.sync if b % 2 == 0 else nc.scalar
                eng.dma_start(
                    send2[b].rearrange("k c r x -> c k r x"), stage2[:, b])
            nc.gpsimd.collective_compute(
                "AllToAll", A.bypass, replica_groups=rg,
                ins=[send2[:].opt()], outs=[recv2[:].opt()])

            # y0 tail fills the A2A#2 window (issued before the transpose so
            # the sync queue doesn't stall behind the recv2 wait)
            for t in range(12, 16):
                y0_iter(t)
            nc.sync.dma_start_transpose(
                inv2[:], recv2[:].rearrange("g k c r x -> (g k c) (r x)"))

        # ---------------- inverse transforms, accumulate into out_d ----------
        with tc.tile_pool(name="psZ", bufs=3, space="PSUM") as psZ_p, \
             tc.tile_pool(name="psO", bufs=4, space="PSUM") as psO_p:
            warm_pe(psZ_p, inv2[:, 0, 0, 0:64])

            def do_psZ(half):
                co = 2 * half
                # 4 concurrent col-tiled matmuls:
                # [Yr(co);Yi(co);Yr(co+1);Yi(co+1)]
                psZ = psZ_p.tile([128, H], F32, tag="psZ",
                                 padded_shape=(None, 512))
                nc.tensor.matmul(psZ[0:32, :], inv2[:, :, :, co],
                                 gh1_sb[:], tile_position=(0, 0))
                nc.tensor.matmul(psZ[32:64, :], inv2[:, :, :, co],
                                 gh2_sb[:], tile_position=(0, 32))
                nc.tensor.matmul(psZ[64:96, :], inv2[:, :, :, co + 1],
                                 gh1_sb[:], tile_position=(0, 64))
                nc.tensor.matmul(psZ[96:128, :], inv2[:, :, :, co + 1],
                                 gh2_sb[:], tile_position=(0, 96))
                z_sb = zpool.tile([128, H], BF, tag="z")
                nc.vector.tensor_copy(z_sb[:], psZ[:])
                return z_sb

            def do_psO(half, z_sb, outt, y0q):
                for p in range(2):
                    slot = 2 * (half % 2) + p
                    psO = psO_p.tile([128, 2, W], F32, tag="psO")
                    for ht in range(2):
                        nc.tensor.matmul(
                            psO[:, ht, :],
                            z_sb[64 * p:64 * p + 64,
                                 128 * ht:128 * ht + 128],
                            cw_sb[64 * p:64 * p + 64, :],
                            tile_position=(64 * p, 0))
                    if p == 0:
                        nc.vector.tensor_add(outt[:, slot], psO[:],
                                             y0q[:, slot])
                    else:
                        nc.scalar.copy(outt[:, slot], psO[:])
                        nc.gpsimd.tensor_add(outt[:, slot], outt[:, slot],
                                             y0q[:, slot])

            outt_tiles = {}

            def get_outt(half):
                q = half // 2
                if q not in outt_tiles:
                    outt = opool.tile([128, 4, 2, W], BF,
                                      tag="outt", name="outt")
                    y0q = opool.tile([128, 4, 2, W], BF, tag="y0q",
                                     name="y0q")
                    nc.scalar.dma_start(
                        y0q[:], y0s[4 * q:4 * q + 4].rearrange(
                            "c (t h) w -> h c t w", t=2))
                    outt_tiles[q] = (outt, y0q)
                return outt_tiles[q]

            def finish_half(half, z_sb):
                outt, y0q = get_outt(half)
                if half + 2 < 64:
                    get_outt(half + 2)  # prefetch next quad's y0
                do_psO(half, z_sb, outt, y0q)
                if half % 2 == 1:
                    q = half // 2
                    outt, y0q = outt_tiles.pop(q)
                    nc.sync.dma_start(
                        out_d[4 * q:4 * q + 4].rearrange(
                            "c (t h) w -> h c t w", t=2),
                        outt[:])

            z_prev = do_psZ(0)
            for half in range(1, 64):
                if half % 12 == 0:
                    warm_pe(psZ_p, inv2[:, 0, 0, 0:64], n=20)
                z_next = do_psZ(half)
                finish_half(half - 1, z_prev)
                z_prev = z_next
            finish_half(63, z_prev)

    nc.compile()
    return nc


_NC_CACHE = {}


def kernel(x, W_lin, w1r, w1i, w2r, w2i):
    x = np.asarray(x)
    FH, FW, GH1, GH2, CW2 = _consts()
    wlt = np.ascontiguousarray(np.asarray(W_lin).T).astype(BF_NP)
    wmix = _wmix_slices(np.asarray(w1r), np.asarray(w1i),
                        np.asarray(w2r), np.asarray(w2i))

    if "nc" not in _NC_CACHE:
        _NC_CACHE["nc"] = _build_nc()
    nc = _NC_CACHE["nc"]

    in_maps = []
    for k in range(NCORES):
        in_maps.append({
            "x": np.ascontiguousarray(x[k]).astype(BF_NP),
            "wlt": wlt, "fh": FH, "fw": FW,
            "gh1": GH1, "gh2": GH2, "cw": CW2,
            "wmix": wmix[k],
        })
    res = run_bass_kernel_spmd(nc, in_maps, list(range(NCORES)))
    out = np.stack([res.results[k]["out"] for k in range(NCORES)], axis=0)
    return out.astype(np.float32)


# revision 16
# speedup vs baseline: 1.3256x; 1.0005x over previous
"""Distributed FNO block on 8 TRN2 NeuronCores — v2.

Strategy: batch-parallel (B=8 -> one batch element per core) for the channel
mixer and both spatial DFTs; mode-parallel (ky sharded, 4 modes per core) for
the spectral channel mixing, with an AllToAll in each direction.

v2 changes vs baseline:
  - y0 (channel mixer) writes straight to out_d; the inverse stage
    accumulates into it with SWDGE dma accum (kills the y0s roundtrip).
  - out_d is bf16 (host casts back to f32).
  - y0 compute is issued AFTER the first AllToAll so it fills the
    collective's dead window on the TensorE queue.
  - wmix is stored host-side in a DMA-contiguous layout and streamed on the
    scalar HWDGE queue, prefetched from early in the kernel.
  - forward W-DFT splits into two PSUM banks so the complex combine reads
    PSUM directly (2 DVE ops/channel instead of 4).
  - inverse processes channel pairs with tile_position packing (full
    128-partition PSUM tiles, concurrent sub-array matmuls).
  - engine rebalance: ACT evacuates PSUM, GpSimd does the mode-mix
    rearranges and collectives, sync/scalar issue all bulk DMA (HWDGE).

Self-contained: shapes/sharding hardcoded, no sibling imports.
"""
import numpy as np
import ml_dtypes
from contextlib import ExitStack

import concourse.bass as bass
import concourse.bacc as bacc
import concourse.tile as tile
from concourse import mybir
from concourse.bass_utils import run_bass_kernel_spmd

B, C, H, W = 8, 128, 256, 256
M0, M1 = 32, 32
NCORES = 8
KX = np.concatenate([np.arange(32), np.arange(H - 32, H)])  # 64 kept kx modes
BF = mybir.dt.bfloat16
F8 = mybir.dt.float8e4
F32 = mybir.dt.float32
BF_NP = ml_dtypes.bfloat16
F8_NP = ml_dtypes.float8_e4m3fn
WSCALE = 2048.0   # wmix scaled into fp8 range
XSCALE = 0.125    # spectrum scaled into fp8 range
# net factor carried by the mix output, removed in the CW constant


# ----------------------------------------------------------------- host consts
def _consts():
    h = np.arange(H)[:, None]
    w = np.arange(W)[:, None]
    ky = np.arange(M1)
    th = 2 * np.pi * h * KX[None, :] / H
    FH = np.concatenate([np.cos(th), -np.sin(th)], axis=1)  # [256, 128]
    tw = 2 * np.pi * w * ky[None, :] / W
    # [cos | -sin | +sin] so Sr/Si accumulate directly in PSUM
    FW = np.concatenate([np.cos(tw), -np.sin(tw), np.sin(tw)], axis=1)  # [256,96]
    thi = 2 * np.pi * np.arange(H)[None, :] * KX[:, None] / H  # [64, 256]
    GH1 = np.concatenate([np.cos(thi) / H, -np.sin(thi) / H], axis=0)  # [128,256]
    GH2 = np.concatenate([np.sin(thi) / H, np.cos(thi) / H], axis=0)   # [128,256]
    twi = 2 * np.pi * ky[:, None] * np.arange(W)[None, :] / W  # [32, 256]
    wt = np.where(ky == 0, 1.0, 2.0)[:, None]
    CW = np.concatenate([wt * np.cos(twi) / W, -wt * np.sin(twi) / W], axis=0)
    CW[32, :] = 0.0  # irfft drops Im(Y[ky=0])
    CW /= (WSCALE * XSCALE)  # undo the fp8 range scaling of the mix path
    # duplicated into both partition halves for the row-tiled inverse-W matmul
    CW2 = np.concatenate([CW, CW], axis=0)  # [128, 256]
    return (FH.astype(BF_NP), FW.astype(BF_NP), GH1.astype(BF_NP),
            GH2.astype(BF_NP), CW2.astype(BF_NP))


def _wmix_slices(w1r, w1i, w2r, w2i):
    """Per-core spectral weight slice, DMA-contiguous for the wblk tiles.

    Layout [4ky, 8kxb, ci, 8kx, 2ri, co] so one 512KB DMA fills a
    [C, 8, 2, C] SBUF tile contiguously per partition."""
    wr = np.concatenate([w1r, w2r], axis=2)  # [ci, co, 64kx, 32ky]
    wi = np.concatenate([w1i, w2i], axis=2)
    wall = np.stack([wr, wi], axis=0)  # [ri, ci, co, kx, ky]
    # -> [ky, kxb, ci, kxi, ri, co]
    wall = wall.reshape(2, C, C, 8, 8, 32)
    wall = wall.transpose(5, 3, 1, 4, 0, 2)  # ky, kxb, ci, kxi, ri, co
    wall = np.ascontiguousarray(wall * WSCALE).astype(F8_NP)
    return [np.ascontiguousarray(wall[4 * k:4 * k + 4]) for k in range(NCORES)]


# ----------------------------------------------------------------- bass kernel
def _build_nc():
    nc = bacc.Bacc(num_devices=NCORES)

    x_d = nc.declare_dram_parameter("x", [C, H, W], BF, isOutput=False)
    wlt_d = nc.declare_dram_parameter("wlt", [C, C], BF, isOutput=False)
    fh_d = nc.declare_dram_parameter("fh", [H, 128], BF, isOutput=False)
    fw_d = nc.declare_dram_parameter("fw", [W, 96], BF, isOutput=False)
    gh1_d = nc.declare_dram_parameter("gh1", [128, H], BF, isOutput=False)
    gh2_d = nc.declare_dram_parameter("gh2", [128, H], BF, isOutput=False)
    cw_d = nc.declare_dram_parameter("cw", [128, W], BF, isOutput=False)
    wmix_d = nc.declare_dram_parameter("wmix", [4, 8, C, 8, 2, C], F8,
                                       isOutput=False)
    out_d = nc.declare_dram_parameter("out", [C, H, W], BF, isOutput=True)

    # internal DRAM
    y0s = nc.dram_tensor("y0s", [C, H, W], BF)
    send1a = nc.dram_tensor("send1a", [8, 64, 4, 2, 64], F8)
    recv1a = nc.dram_tensor("recv1a", [8, 64, 4, 2, 64], F8)
    send1b = nc.dram_tensor("send1b", [8, 64, 4, 2, 64], F8)
    recv1b = nc.dram_tensor("recv1b", [8, 64, 4, 2, 64], F8)
    send2 = nc.dram_tensor("send2", [8, 4, C, 2, 64], BF)
    recv2 = nc.dram_tensor("recv2", [8, 4, C, 2, 64], BF)

    rg = [list(range(NCORES))]
    A = mybir.AluOpType

    with tile.TileContext(nc) as tc, ExitStack() as ctx:
        cpool = ctx.enter_context(tc.tile_pool(name="consts", bufs=1))
        spool = ctx.enter_context(tc.tile_pool(name="stages", bufs=1))
        xqpool = ctx.enter_context(tc.tile_pool(name="xq", bufs=6))
        xhpool = ctx.enter_context(tc.tile_pool(name="xh", bufs=6))
        ypool = ctx.enter_context(tc.tile_pool(name="y", bufs=3))
        wpool = ctx.enter_context(tc.tile_pool(name="wmix", bufs=6))
        zpool = ctx.enter_context(tc.tile_pool(name="z", bufs=4))
        opool = ctx.enter_context(tc.tile_pool(name="o", bufs=4))

        # constants into SBUF
        fh_sb = [cpool.tile([128, 128], BF, tag=f"fh{t}", name=f"fh{t}")
                 for t in range(2)]
        fw_sb = [cpool.tile([128, 96], BF, tag=f"fw{t}", name=f"fw{t}")
                 for t in range(2)]
        for t in range(2):
            nc.sync.dma_start(fh_sb[t][:], fh_d[128 * t:128 * (t + 1), :])
            nc.sync.dma_start(fw_sb[t][:], fw_d[128 * t:128 * (t + 1), :])
        wlt_sb = cpool.tile([C, C], BF, tag="wlt")
        nc.scalar.dma_start(wlt_sb[:], wlt_d[:])
        gh1_sb = cpool.tile([128, H], BF, tag="gh1")
        gh2_sb = cpool.tile([128, H], BF, tag="gh2")
        cw_sb = cpool.tile([128, W], BF, tag="cw")
        nc.scalar.dma_start(gh1_sb[:], gh1_d[:])
        nc.scalar.dma_start(gh2_sb[:], gh2_d[:])
        nc.scalar.dma_start(cw_sb[:], cw_d[:])

        # big staging tiles
        stage1 = spool.tile([64, C, 64], F8, tag="stage1")  # [(ri ky), c, kx]
        mm_in = spool.tile([C, 8, 4, 2, 64], F8, tag="mm_in")  # A2A#1 recv
        rhs2s = spool.tile([C, 4, 64, 2, 8], F8, tag="rhs2s")  # [-Xi | Xr]
        stage2 = spool.tile([C, 8, 4, 2, 64], BF, tag="stage2")  # modemix out
        inv2 = spool.tile([128, 8, 4, C], BF, tag="inv2")      # A2A#2 recv

        # wmix streaming: 6 blocks prefetch upfront (SWDGE, gpsimd queue),
        # the rest are issued just-in-time inside the mix loop so pool
        # recycling never stalls a queue that later phases depend on.
        wblk_tiles = [None] * 32

        def issue_wblk(i):
            kyi, kxb = divmod(i, 8)
            t = wpool.tile([C, 8, 2, C], F8, tag="wblk", name="wblk")
            nc.scalar.dma_start(t[:], wmix_d[kyi, kxb])
            wblk_tiles[i] = t

        for i in range(6):
            issue_wblk(i)

        # PE warm-up: a dense dummy-matmul burst (~4us) flips the HAM clock
        # gate to 8/8 so the phase that follows runs at 2.4 GHz not 1.2.
        def warm_pe(pool, dep_ap, n=48):
            ps = pool.tile([64, 64], F32, tag="warm", name="warm",
                           padded_shape=(None, 512), bufs=1)
            for _ in range(n):
                nc.tensor.matmul(ps[:], dep_ap, fh_sb[0][:, 0:64],
                                 start=True, stop=True)

        # ---------------- forward truncated DFT (per channel) ----------------
        with tc.tile_pool(name="psA", bufs=4, space="PSUM") as psA_p, \
             tc.tile_pool(name="psW", bufs=3, space="PSUM") as psW_p:
            warm_pe(psW_p, fh_sb[1][:, 0:64])
            for blk in range(32):
                xq = xqpool.tile([128, 4, 2, 256], BF, tag="xq", name="xq")
                nc.sync.dma_start(
                    xq[:], x_d[4 * blk:4 * blk + 4].rearrange(
                        "c (t h) w -> h c t w", t=2))
                for ci in range(4):
                    c = 4 * blk + ci
                    xh = [None, None]
                    for wt_i in range(2):
                        ps = psA_p.tile([128, 128], F32, tag="psA",
                                        padded_shape=(None, 512))
                        for ht in range(2):
                            nc.tensor.matmul(
                                ps[:],
                                xq[:, ci, ht, 128 * wt_i:128 * (wt_i + 1)],
                                fh_sb[ht][:], start=(ht == 0), stop=(ht == 1))
                        xh[wt_i] = xhpool.tile([128, 128], BF, tag="xh",
                                               name="xh")
                        if wt_i == 0:
                            nc.scalar.copy(xh[wt_i][:], ps[:])
                        else:
                            nc.vector.tensor_copy(xh[wt_i][:], ps[:])
                    # Sr = cos@Xr + sin@Xi (rows 0:32), Si = cos@Xi - sin@Xr
                    # (rows 32:64), accumulated in PSUM over both w-halves.
                    psRI = psW_p.tile([64, 64], F32, tag="psRI", name="psRI",
                                      padded_shape=(None, 512))
                    for wt_i in range(2):
                        st = wt_i == 0
                        sp = wt_i == 1
                        xr = xh[wt_i][:, 0:64]
                        xi = xh[wt_i][:, 64:128]
                        fw = fw_sb[wt_i]
                        nc.tensor.matmul(psRI[0:32, :], fw[:, 0:32], xr,
                                         start=st, stop=False,
                                         tile_position=(0, 0))
                        nc.tensor.matmul(psRI[32:64, :], fw[:, 0:32], xi,
                                         start=st, stop=False,
                                         tile_position=(0, 32))
                        nc.tensor.matmul(psRI[0:32, :], fw[:, 64:96], xi,
                                         start=False, stop=sp,
                                         tile_position=(0, 0))
                        nc.tensor.matmul(psRI[32:64, :], fw[:, 32:64], xr,
                                         start=False, stop=sp,
                                         tile_position=(0, 32))
                    nc.vector.tensor_scalar_mul(stage1[:, c, :],
                                                psRI[:], XSCALE)
                if blk == 15:
                    # first 64 channels done: ship them while fwd continues
                    for g in range(8):
                        for r in range(2):
                            eng = nc.sync if g % 2 == 0 else nc.scalar
                            eng.dma_start(
                                send1a[g][:, :, r, :].rearrange(
                                    "c k x -> k c x"),
                                stage1[32 * r + 4 * g:32 * r + 4 * g + 4,
                                       0:64, :])
                    nc.gpsimd.collective_compute(
                        "AllToAll", A.bypass, replica_groups=rg,
                        ins=[send1a[:].opt()], outs=[recv1a[:].opt()])

        # A2A #1b: remaining channels
        for g in range(8):
            for r in range(2):
                eng = nc.sync if g % 2 == 0 else nc.scalar
                eng.dma_start(
                    send1b[g][:, :, r, :].rearrange("c k x -> k c x"),
                    stage1[32 * r + 4 * g:32 * r + 4 * g + 4, 64:128, :])
        nc.gpsimd.collective_compute(
            "AllToAll", A.bypass, replica_groups=rg,
            ins=[send1b[:].opt()], outs=[recv1b[:].opt()])

        nc.sync.dma_start(mm_in[0:64, :],
                          recv1a[:].rearrange("b c k r x -> c b k r x"))
        nc.sync.dma_start(mm_in[64:128, :],
                          recv1b[:].rearrange("b c k r x -> c b k r x"))
        # rhs2s[:, k, x, 0, b] = -Xi, rhs2s[:, k, x, 1, b] = Xr (per-kyi chunks
        # so the mix can start after the first); rhs1 is read via a strided
        # view of mm_in directly in the matmul.
        for kyi in range(4):
            nc.vector.tensor_scalar_mul(
                rhs2s[:, kyi, :, 0, :],
                mm_in[:, :, kyi, 1, :].rearrange("c b x -> c x b"), -1.0)
            nc.vector.tensor_copy(
                rhs2s[:, kyi, :, 1, :],
                mm_in[:, :, kyi, 0, :].rearrange("c b x -> c x b"))

        # ---------------- y0 = W_lin @ x -> out_d (fills A2A#1 window) -------
        with tc.tile_pool(name="psy", bufs=2, space="PSUM") as psy_p, \
             tc.tile_pool(name="psm", bufs=4, space="PSUM") as psm_p:
            def y0_iter(t):
                xt = ypool.tile([C, 16, W], BF, tag="yx", name="yx")
                nc.scalar.dma_start(xt[:], x_d[:, 16 * t:16 * t + 16, :])
                y0t = ypool.tile([C, 16, W], BF, tag="y0t", name="y0t")
                for j in range(8):
                    psy = psy_p.tile([C, 2, W], F32, tag="psy")
                    nc.tensor.matmul(psy[:], wlt_sb[:],
                                     xt[:, 2 * j:2 * j + 2, :])
                    if j % 2 == 0:
                        nc.vector.tensor_copy(y0t[:, 2 * j:2 * j + 2, :],
                                              psy[:])
                    else:
                        nc.scalar.copy(y0t[:, 2 * j:2 * j + 2, :], psy[:])
                nc.sync.dma_start(y0s[:, 16 * t:16 * t + 16, :], y0t[:])

            for t in range(12):
                y0_iter(t)

            # ---------------- modemix (ky-sharded, all batches) --------------
            warm_pe(psm_p, mm_in[:, 0, 0, 0, 0:64])
            for kyi in range(4):
                if kyi > 0:
                    warm_pe(psm_p, mm_in[:, 0, 0, 0, 0:64], n=20)
                for kxb in range(8):
                    i = 8 * kyi + kxb
                    if i + 6 < 32:
                        issue_wblk(i + 6)
                    wblk = wblk_tiles[i]
                    wblk_tiles[i] = None
                    for half in range(2):
                        psm = psm_p.tile([C, 4, 2, 8], F32, tag="psm",
                                         padded_shape=(None, None, None, 64))
                        for kxi in range(4):
                            slot = 4 * half + kxi
                            kx = 8 * kxb + slot
                            nc.tensor.matmul(
                                psm[:, kxi], wblk[:, slot, 0, :],
                                mm_in[:, :, kyi, :, kx].rearrange(
                                    "c b r -> c r b"),
                                start=True, stop=False)
                            nc.tensor.matmul(psm[:, kxi], wblk[:, slot, 1, :],
                                             rhs2s[:, kyi, kx, :, :],
                                             start=False, stop=True)
                        kx0 = 8 * kxb + 4 * half
                        nc.vector.tensor_copy(
                            stage2[:, :, kyi, :, kx0:kx0 + 4],
                            psm[:].rearrange("c x r b -> c b r x"))

            # A2A #2: back to batch-sharded full spectrum
            for b in range(8):
                eng = nc.sync if b % 2 == 0 else nc.scalar
                eng.dma_start(
                    send2[b].rearrange("k c r x -> c k r x"), stage2[:, b])
            nc.gpsimd.collective_compute(
                "AllToAll", A.bypass, replica_groups=rg,
                ins=[send2[:].opt()], outs=[recv2[:].opt()])

            # y0 tail fills the A2A#2 window (issued before the transpose so
            # the sync queue doesn't stall behind the recv2 wait)
            for t in range(12, 16):
                y0_iter(t)
            nc.sync.dma_start_transpose(
                inv2[:], recv2[:].rearrange("g k c r x -> (g k c) (r x)"))

        # ---------------- inverse transforms, accumulate into out_d ----------
        with tc.tile_pool(name="psZ", bufs=3, space="PSUM") as psZ_p, \
             tc.tile_pool(name="psO", bufs=4, space="PSUM") as psO_p:
            warm_pe(psZ_p, inv2[:, 0, 0, 0:64])

            def do_psZ(half):
                co = 2 * half
                # 4 concurrent col-tiled matmuls:
                # [Yr(co);Yi(co);Yr(co+1);Yi(co+1)]
                psZ = psZ_p.tile([128, H], F32, tag="psZ",
                                 padded_shape=(None, 512))
                nc.tensor.matmul(psZ[0:32, :], inv2[:, :, :, co],
                                 gh1_sb[:], tile_position=(0, 0))
                nc.tensor.matmul(psZ[32:64, :], inv2[:, :, :, co],
                                 gh2_sb[:], tile_position=(0, 32))
                nc.tensor.matmul(psZ[64:96, :], inv2[:, :, :, co + 1],
                                 gh1_sb[:], tile_position=(0, 64))
                nc.tensor.matmul(psZ[96:128, :], inv2[:, :, :, co + 1],
                                 gh2_sb[:], tile_position=(0, 96))
                z_sb = zpool.tile([128, H], BF, tag="z")
                nc.vector.tensor_copy(z_sb[:], psZ[:])
                return z_sb

            def do_psO(half, z_sb, outt, y0q):
                for p in range(2):
                    slot = 2 * (half % 2) + p
                    psO = psO_p.tile([128, 2, W], F32, tag="psO")
                    for ht in range(2):
                        nc.tensor.matmul(
                            psO[:, ht, :],
                            z_sb[64 * p:64 * p + 64,
                                 128 * ht:128 * ht + 128],
                            cw_sb[64 * p:64 * p + 64, :],
                            tile_position=(64 * p, 0))
                    if p == 0:
                        nc.vector.tensor_add(outt[:, slot], psO[:],
                                             y0q[:, slot])
                    else:
                        nc.scalar.copy(outt[:, slot], psO[:])
                        nc.gpsimd.tensor_add(outt[:, slot], outt[:, slot],
                                             y0q[:, slot])

            outt_tiles = {}

            def get_outt(half):
                q = half // 2
                if q not in outt_tiles:
                    outt = opool.tile([128, 4, 2, W], BF,
                                      tag="outt", name="outt")
                    y0q = opool.tile([128, 4, 2, W], BF, tag="y0q",
                                     name="y0q")
                    nc.scalar.dma_start(
                        y0q[:], y0s[4 * q:4 * q + 4].rearrange(
                            "c (t h) w -> h c t w", t=2))
                    outt_tiles[q] = (outt, y0q)
                return outt_tiles[q]

            def finish_half(half, z_sb):
                outt, y0q = get_outt(half)
                if half + 2 < 64:
                    get_outt(half + 2)  # prefetch next quad's y0
                do_psO(half, z_sb, outt, y0q)
                if half % 2 == 1:
                    q = half // 2
                    outt, y0q = outt_tiles.pop(q)
                    nc.sync.dma_start(
                        out_d[4 * q:4 * q + 4].rearrange(
                            "c (t h) w -> h c t w", t=2),
                        outt[:])

            zq = [do_psZ(0), do_psZ(1)]
            for half in range(2, 64):
                if half % 8 == 0:
                    warm_pe(psZ_p, inv2[:, 0, 0, 0:64], n=20)
                zq.append(do_psZ(half))
                finish_half(half - 2, zq.pop(0))
            finish_half(62, zq[0])
            finish_half(63, zq[1])

    nc.compile()
    return nc


_NC_CACHE = {}


def kernel(x, W_lin, w1r, w1i, w2r, w2i):
    x = np.asarray(x)
    FH, FW, GH1, GH2, CW2 = _consts()
    wlt = np.ascontiguousarray(np.asarray(W_lin).T).astype(BF_NP)
    wmix = _wmix_slices(np.asarray(w1r), np.asarray(w1i),
                        np.asarray(w2r), np.asarray(w2i))

    if "nc" not in _NC_CACHE:
        _NC_CACHE["nc"] = _build_nc()
    nc = _NC_CACHE["nc"]

    in_maps = []
    for k in range(NCORES):
        in_maps.append({
            "x": np.ascontiguousarray(x[k]).astype(BF_NP),
            "wlt": wlt, "fh": FH, "fw": FW,
            "gh1": GH1, "gh2": GH2, "cw": CW2,
            "wmix": wmix[k],
        })
    res = run_bass_kernel_spmd(nc, in_maps, list(range(NCORES)))
    out = np.stack([res.results[k]["out"] for k in range(NCORES)], axis=0)
    return out.astype(np.float32)
